# revision 1
# baseline (speedup 1.0000x reference)
"""Multi-modality double-value attention on 8 TRN2 NeuronCores.

Sharding: data-parallel over batch (16 items -> 2 per core). Each core runs
the full attention block for its 2 items; weights are replicated. No
collectives. Host pre-transposes x to x^T and casts inputs to bf16; compute
is bf16 with fp32 PSUM accumulation; output is fp32.
"""

import numpy as np
import ml_dtypes

B, N, C = 16, 906, 768
H = 12
D = 64
M1 = 513
N_CORES = 8
BPC = B // N_CORES          # batch items per core
KC = C // 128               # 6 contraction chunks over C
NPAIR = H // 2              # 6 head pairs
NCH = (N + 127) // 128      # 8 key/token chunks over N
KCH = [(i * 128, min(128, N - i * 128)) for i in range(NCH)]
QP = [(0, 512), (512, N - 512)]      # column passes over N
CPASS = [(0, 512), (512, C - 512)]   # column passes over C
SCALE = D ** -0.5
PW = 194  # per-head-pair value block: [V_e(64) | 1 | 1 | 1 | 0*63 | V_o(64)]

TRACE = False          # set by test.py to capture a HW profile
DEBUG_DUMP = False     # add intermediate DRAM outputs (denominators, recips, oT)
LAST_RESULTS = None    # BassKernelResults of the most recent run

_BUILT = None


def _install_trace_shim():
    """The image's antenv lacks axon_hooks; recreate it so trace=True works."""
    import sys, types
    if "antenv.axon_hooks" in sys.modules:
        return
    mod = types.ModuleType("antenv.axon_hooks")
    mod._hook = None
    mod.set_axon_ntff_profile_hook = lambda h: setattr(mod, "_hook", h)
    mod.get_axon_ntff_profile_hook = lambda: mod._hook
    sys.modules["antenv.axon_hooks"] = mod
    import antenv
    antenv.axon_hooks = mod
    from trn_agent_boot.trn_boot import _ntff_profile_via_ctypes
    mod.set_axon_ntff_profile_hook(_ntff_profile_via_ctypes("/opt/axon/libaxon_pjrt.so"))


def _build():
    import concourse.tile as tile
    from concourse import bacc, mybir

    BF = mybir.dt.bfloat16
    F32 = mybir.dt.float32
    AF = mybir.ActivationFunctionType

    nc = bacc.Bacc("TRN2", target_bir_lowering=False, debug=False, num_devices=N_CORES)

    xT_d = nc.dram_tensor("xT", [BPC, C, N], BF, kind="ExternalInput").ap()
    w_d = {
        wn: nc.dram_tensor(wn, [C, C], BF, kind="ExternalInput").ap()
        for wn in ("wq", "wk", "wv", "wvc", "wp")
    }
    bias_d = nc.dram_tensor("bias", [128, C], F32, kind="ExternalInput").ap()
    out_d = nc.dram_tensor("out", [BPC, N, C], F32, kind="ExternalOutput").ap()
    if DEBUG_DUMP:
        dbg_rc = nc.dram_tensor("dbg_rc", [BPC, H, N], F32, kind="ExternalOutput").ap()
        dbg_ot = nc.dram_tensor("dbg_ot", [BPC, NPAIR, 128, N], BF, kind="ExternalOutput").ap()
        dbg_e = nc.dram_tensor("dbg_e", [BPC, 2, NCH, 128, N], BF, kind="ExternalOutput").ap()
        dbg_t1 = nc.dram_tensor("dbg_t1", [BPC, NPAIR, 2, 128, 512], F32, kind="ExternalOutput").ap()
        dbg_v = nc.dram_tensor("dbg_v", [BPC, 3, NCH, 128, NPAIR * PW], BF, kind="ExternalOutput").ap()

    with tile.TileContext(nc) as tc:
        from contextlib import ExitStack
        from concourse import library_config

        with ExitStack() as ctx:
            wpool = ctx.enter_context(tc.tile_pool(name="wpool", bufs=1))
            sb = ctx.enter_context(tc.tile_pool(name="sb", bufs=1))
            ps = ctx.enter_context(tc.tile_pool(name="ps", bufs=1, space="PSUM"))

            # partition_broadcast lives in the gpsimd 'attn' library; the
            # default 'standard' library executes it as garbage on HW
            nc.gpsimd.load_library(library_config.attn)

            # ---- constants: weights + bias ----
            w_sb = {}
            for wn in ("wq", "wk", "wv", "wvc", "wp"):
                tiles = []
                for kc in range(KC):
                    t = wpool.tile([128, C], BF, name=f"{wn}_{kc}", tag=f"{wn}_{kc}")
                    nc.sync.dma_start(t[:], w_d[wn][kc * 128:(kc + 1) * 128, :])
                    tiles.append(t)
                w_sb[wn] = tiles
            bias_sb = wpool.tile([128, C], F32, name="bias_sb", tag="bias_sb")
            nc.sync.dma_start(bias_sb[:], bias_d[:])

            # ---- x^T tiles, both items prefetched ----
            xT = {}
            for it in range(BPC):
                for kc in range(KC):
                    t = sb.tile([128, N], BF, name=f"xT_{it}_{kc}", tag="xT", bufs=6)
                    nc.sync.dma_start(t[:], xT_d[it, kc * 128:(kc + 1) * 128, :])
                    xT[(it, kc)] = t

            for it in range(BPC):
                # ============ projections ============
                qT, kTh = [], []
                for t_ in range(NPAIR):
                    dst = sb.tile([128, N], BF, name=f"qT_{it}_{t_}",
                                  tag="qT", bufs=NPAIR + 1)
                    for (qs, qw) in QP:
                        pp = ps.tile([128, 512], F32, name="pp", tag="ps_mm", bufs=4)
                        with tc.tile_critical():
                            for kc in range(KC):
                                nc.tensor.matmul(
                                    pp[:, 0:qw],
                                    lhsT=w_sb["wq"][kc][:, t_ * 128:(t_ + 1) * 128],
                                    rhs=xT[(it, kc)][:, qs:qs + qw],
                                    start=(kc == 0), stop=(kc == KC - 1),
                                )
                        nc.scalar.copy(dst[:, qs:qs + qw], pp[:, 0:qw])
                    qT.append(dst)
                    # k^T per head, zero-padded to 128 partitions so S^T runs as a
                    # plain K=128 matmul (no PE row tiling -- T8 tile corrupts on HW)
                    ke = sb.tile([128, N], BF, name=f"kTh_{it}_{2*t_}", tag="kT", bufs=2 * NPAIR + 1)
                    ko = sb.tile([128, N], BF, name=f"kTh_{it}_{2*t_+1}", tag="kT", bufs=2 * NPAIR + 1)
                    nc.vector.memset(ke[64:128, :], 0.0)
                    nc.vector.memset(ko[0:64, :], 0.0)
                    for (qs, qw) in QP:
                        pp = ps.tile([128, 512], F32, name="pp", tag="ps_mm", bufs=4)
                        with tc.tile_critical():
                            for kc in range(KC):
                                nc.tensor.matmul(
                                    pp[:, 0:qw],
                                    lhsT=w_sb["wk"][kc][:, t_ * 128:(t_ + 1) * 128],
                                    rhs=xT[(it, kc)][:, qs:qs + qw],
                                    start=(kc == 0), stop=(kc == KC - 1),
                                )
                        nc.scalar.copy(ke[0:64, qs:qs + qw], pp[0:64, 0:qw])
                        nc.scalar.copy(ko[64:128, qs:qs + qw], pp[64:128, 0:qw])
                    kTh.append(ke)
                    kTh.append(ko)

                v_sb, vc_sb = [], []
                for c, (ts, tsz) in enumerate(KCH):
                    for dst_list, wn, tg in ((v_sb, "wv", "v"), (vc_sb, "wvc", "vc")):
                        dst = sb.tile([128, NPAIR * PW], BF, name=f"{tg}_{it}_{c}",
                                      tag=tg, bufs=NCH + 1)
                        if tsz < 128:
                            # stationary loads may read all 128 partitions; keep
                            # the unwritten tail finite
                            nc.vector.memset(dst[:, :], 0.0)
                        dvw = dst[0:tsz, :].rearrange("p (g c) -> p g c", c=PW)
                        for (cs, cw) in CPASS:
                            pp = ps.tile([128, 512], F32, name="pp", tag="ps_mm", bufs=4)
                            with tc.tile_critical():
                                for kc in range(KC):
                                    nc.tensor.matmul(
                                        pp[0:tsz, 0:cw],
                                        lhsT=xT[(it, kc)][:, ts:ts + tsz],
                                        rhs=w_sb[wn][kc][:, cs:cs + cw],
                                        start=(kc == 0), stop=(kc == KC - 1),
                                    )
                            g0, gn = (0, 4) if cs == 0 else (4, 2)
                            src = pp[0:tsz, 0:cw].rearrange("p (g r d) -> p g r d", r=2, d=D)
                            nc.scalar.copy(dvw[:, g0:g0 + gn, 0:D], src[:, :, 0, :])
                            nc.scalar.copy(dvw[:, g0:g0 + gn, 130:194], src[:, :, 1, :])
                        nc.vector.memset(dvw[:, :, 64:67], 1.0)
                        nc.vector.memset(dvw[:, :, 67:130], 0.0)
                        dst_list.append(dst)

                # mixed tiles for the key chunk straddling M1 (chunk 4: keys 512..639)
                amix = sb.tile([128, NPAIR * PW], BF, name=f"amix_{it}", tag="amix", bufs=BPC)
                vmix = sb.tile([128, NPAIR * PW], BF, name=f"vmix_{it}", tag="vmix", bufs=BPC)
                nc.vector.tensor_copy(amix[:, :], vc_sb[4][:, :])
                nc.vector.tensor_copy(amix[0:1, :], v_sb[4][0:1, :])
                nc.vector.tensor_copy(vmix[:, :], v_sb[4][:, :])
                nc.vector.tensor_copy(vmix[0:1, :], vc_sb[4][0:1, :])
                if DEBUG_DUMP:
                    for c_ in range(NCH):
                        nc.sync.dma_start(dbg_v[it, 0, c_], v_sb[c_][:, :])
                        nc.sync.dma_start(dbg_v[it, 1, c_], vc_sb[c_][:, :])
                    nc.sync.dma_start(dbg_v[it, 2, 0], amix[:, :])
                    nc.sync.dma_start(dbg_v[it, 2, 1], vmix[:, :])

                # ============ attention, one head pair at a time ============
                oT = []
                for p in range(NPAIR):
                    # S^T = scores transposed (keys on partitions), then exp
                    exps = {}
                    for c, (ks, ksz) in enumerate(KCH):
                        for par in range(2):
                            # exp split into 4-byte-aligned tiles: eA = q[0,512),
                            # eR = q=512, eB = q[513,906)
                            eA = sb.tile([128, 512], BF, name="eA", tag="expA", bufs=17)
                            eB = sb.tile([128, 394], BF, name="eB", tag="expB", bufs=17)
                            eR = sb.tile([128, 2], BF, name="eR", tag="expR", bufs=17)
                            pst = ps.tile([128, 512], F32, name="pst", tag="ps_s", bufs=4)
                            nc.tensor.matmul(pst[0:ksz, 0:512],
                                             lhsT=kTh[2 * p + par][:, ks:ks + ksz],
                                             rhs=qT[p][:, 0:512], start=True, stop=True)
                            nc.scalar.activation(eA[0:ksz, :], pst[0:ksz, 0:512],
                                                 AF.Exp, scale=SCALE)
                            pst2 = ps.tile([128, 512], F32, name="pst2", tag="ps_s", bufs=4)
                            nc.tensor.matmul(pst2[0:ksz, 0:394],
                                             lhsT=kTh[2 * p + par][:, ks:ks + ksz],
                                             rhs=qT[p][:, 512:906], start=True, stop=True)
                            nc.scalar.activation(eR[0:ksz, 0:1], pst2[0:ksz, 0:1],
                                                 AF.Exp, scale=SCALE)
                            nc.scalar.activation(eB[0:ksz, 0:393], pst2[0:ksz, 1:394],
                                                 AF.Exp, scale=SCALE)
                            exps[(c, par)] = (eA, eR, eB)

                    ot = sb.tile([128, N], BF, name=f"oT_{it}_{p}", tag="oT", bufs=NPAIR + 1)
                    for par in range(2):
                        # three accumulators, each a single bank holding a single
                        # accumulation group: q[0,512) a-mod, q=512 a-mod, q[513,906) v-mod
                        t1 = ps.tile([128, 512], F32, name="t1", tag="ps_mm", bufs=4)
                        t2 = ps.tile([128, 512], F32, name="t2", tag="ps_mm", bufs=4)
                        if par == 0:
                            o1, o2 = t1[0:65, 0:512], t2[0:65, 0:393]
                            mrows = slice(0, 65)
                            csl = slice(p * PW, p * PW + 65)          # [V_even | 1]
                            drow, orows = 64, slice(0, 64)
                        else:
                            o1, o2 = t1[:, 0:512], t2[:, 0:393]
                            mrows = slice(0, 128)
                            csl = slice(p * PW + 66, p * PW + PW)     # [1 | 0*63 | V_odd]
                            drow, orows = 0, slice(64, 128)
                        # modality-a queries: q in [0,512) -> o1
                        with tc.tile_critical():
                            for c, (ks, ksz) in enumerate(KCH):
                                va = amix if c == 4 else (v_sb[c] if c < 4 else vc_sb[c])
                                nc.tensor.matmul(o1, lhsT=va[0:ksz, csl],
                                                 rhs=exps[(c, par)][0][0:ksz, 0:512],
                                                 start=(c == 0), stop=(c == NCH - 1),
                                                 tile_position=(0, 0))
                        # a-modality q=512 column: 8 independent single-matmul
                        # writes (start&stop each), accumulated on the DVE --
                        # avoids a long-lived 1-wide PSUM accumulation group
                        racc = sb.tile([128, 4], F32, name="racc", tag="racc", bufs=2)
                        nc.vector.memset(racc[:, 0:1], 0.0)
                        for c, (ks, ksz) in enumerate(KCH):
                            va = amix if c == 4 else (v_sb[c] if c < 4 else vc_sb[c])
                            rc1 = ps.tile([128, 512], F32, name="rc1", tag="ps_s", bufs=4)
                            nc.tensor.matmul(rc1[mrows, 0:1], lhsT=va[0:ksz, csl],
                                             rhs=exps[(c, par)][1][0:ksz, 0:1],
                                             start=True, stop=True, tile_position=(0, 0))
                            nc.vector.tensor_add(racc[mrows, 0:1], racc[mrows, 0:1],
                                                 rc1[mrows, 0:1])
                        # modality-v queries: q in [513,906) -> o2
                        with tc.tile_critical():
                            for c, (ks, ksz) in enumerate(KCH):
                                vv = vmix if c == 4 else (vc_sb[c] if c < 4 else v_sb[c])
                                nc.tensor.matmul(o2, lhsT=vv[0:ksz, csl],
                                                 rhs=exps[(c, par)][2][0:ksz, 0:393],
                                                 start=(c == 0), stop=(c == NCH - 1),
                                                 tile_position=(0, 0))
                        # softmax division: denominators sit in row `drow`
                        bcs = sb.tile([128, N], F32, name="bcs", tag="bc", bufs=2)
                        nc.vector.reciprocal(bcs[drow:drow + 1, 0:512], t1[drow:drow + 1, 0:512])
                        nc.vector.reciprocal(bcs[drow:drow + 1, 512:513], racc[drow:drow + 1, 0:1])
                        nc.vector.reciprocal(bcs[drow:drow + 1, 513:906], t2[drow:drow + 1, 0:393])
                        if drow != 0:
                            # hw partition_broadcast reads physical partition 0;
                            # relocate the reciprocal row there first
                            nc.sync.dma_start(bcs[0:1, 0:906], bcs[drow:drow + 1, 0:906])
                        bc2 = sb.tile([128, N], F32, name="bc2", tag="bc2", bufs=2)
                        nc.gpsimd.partition_broadcast(bc2[:, 0:906], bcs[0:1, 0:906])
                        nc.vector.tensor_mul(ot[orows, 0:512], t1[orows, 0:512], bc2[orows, 0:512])
                        nc.vector.tensor_mul(ot[orows, 512:513], racc[orows, 0:1], bc2[orows, 512:513])
                        nc.vector.tensor_mul(ot[orows, 513:906], t2[orows, 0:393], bc2[orows, 513:906])
                        if DEBUG_DUMP:
                            h = 2 * p + par
                            nc.sync.dma_start(dbg_rc[it, h, 0:512], bc[drow:drow + 1, 0:512])
                            nc.sync.dma_start(dbg_rc[it, h, 512:906], bc[drow:drow + 1, 512:906])
                            t1c = sb.tile([128, 512], F32, name="t1c", tag="t1c", bufs=2)
                            nc.vector.tensor_copy(t1c[:, :], t1[:, :])
                            nc.sync.dma_start(dbg_t1[it, p, par], t1c[:, :])
                    if DEBUG_DUMP:
                        nc.sync.dma_start(dbg_ot[it, p], ot[:, :])
                        if p == 1:
                            for par_ in range(2):
                                for c_ in range(NCH):
                                    nc.sync.dma_start(dbg_e[it, par_, c_, :, 0:512], exps[(c_, par_)][0][:, :])
                    oT.append(ot)

                # ============ output projection + bias ============
                for c, (ts, tsz) in enumerate(KCH):
                    for (cs, cw) in CPASS:
                        pp = ps.tile([128, 512], F32, name="pp", tag="ps_mm", bufs=4)
                        with tc.tile_critical():
                            for kp in range(NPAIR):
                                nc.tensor.matmul(
                                    pp[0:tsz, 0:cw],
                                    lhsT=oT[kp][:, ts:ts + tsz],
                                    rhs=w_sb["wp"][kp][:, cs:cs + cw],
                                    start=(kp == 0), stop=(kp == NPAIR - 1),
                                )
                        ob = sb.tile([128, 512], F32, name="ob", tag="ob", bufs=2)
                        nc.vector.tensor_add(ob[0:tsz, 0:cw], pp[0:tsz, 0:cw],
                                             bias_sb[0:tsz, cs:cs + cw])
                        nc.sync.dma_start(out_d[it, ts:ts + tsz, cs:cs + cw], ob[0:tsz, 0:cw])

    nc.compile()
    return nc


def _get_built():
    global _BUILT
    if _BUILT is None:
        _BUILT = _build()
    return _BUILT


def kernel(x, Wq, Wk, Wv, Wvc, Wp, bp):
    global LAST_RESULTS
    from concourse.bass_utils import run_bass_kernel_spmd

    x = np.asarray(x, dtype=np.float32)
    bf = ml_dtypes.bfloat16
    xT = np.ascontiguousarray(x.transpose(0, 2, 1)).astype(bf)      # (B, C, N)
    ws = {
        "wq": np.asarray(Wq, dtype=np.float32).astype(bf),
        "wk": np.asarray(Wk, dtype=np.float32).astype(bf),
        "wv": np.asarray(Wv, dtype=np.float32).astype(bf),
        "wvc": np.asarray(Wvc, dtype=np.float32).astype(bf),
        "wp": np.asarray(Wp, dtype=np.float32).astype(bf),
    }
    bias = np.ascontiguousarray(
        np.broadcast_to(np.asarray(bp, dtype=np.float32), (128, C))
    )

    if TRACE:
        _install_trace_shim()

    nc = _get_built()
    in_maps = []
    for i in range(N_CORES):
        m = {"xT": np.ascontiguousarray(xT[i * BPC:(i + 1) * BPC]), "bias": bias}
        m.update(ws)
        in_maps.append(m)

    res = run_bass_kernel_spmd(nc, in_maps, list(range(N_CORES)), trace=TRACE,
                               stitch_traces=False)
    LAST_RESULTS = res
    out = np.concatenate([res.results[i]["out"] for i in range(N_CORES)], axis=0)
    return out



# revision 2
# speedup vs baseline: 2.2131x; 2.2131x over previous
"""Multi-modality double-value attention on 8 TRN2 NeuronCores.

Sharding: data-parallel over batch (16 items -> 2 per core). Each core runs
the full attention block for its 2 items; weights are replicated. No
collectives. Host pre-transposes x to x^T and casts inputs to bf16; compute
is bf16 with fp32 PSUM accumulation; output is fp32.

v2 pipeline: the two items are software-pipelined so the PE never idles long
enough for the HAM clock gate to re-throttle. Scores for one (pair, parity)
go into a single 2-bank PSUM tile and are exponentiated with one 906-col
ACT instruction; attention outputs are evacuated to SBUF immediately
(unnormalized, bf16) so PSUM banks recycle fast; softmax division happens
late via one batched reciprocal per item + gpsimd row-broadcasts.
"""

import numpy as np
import ml_dtypes

B, N, C = 16, 906, 768
H = 12
D = 64
M1 = 513
N_CORES = 8
BPC = B // N_CORES          # batch items per core
KC = C // 128               # 6 contraction chunks over C
NPAIR = H // 2              # 6 head pairs
NCH = (N + 127) // 128      # 8 key/token chunks over N
KCH = [(i * 128, min(128, N - i * 128)) for i in range(NCH)]
QP = [(0, 512), (512, N - 512)]      # column passes over N
CPASS = [(0, 512), (512, C - 512)]   # column passes over C
SCALE = D ** -0.5
PW = 194  # per-head-pair value block: [V_e(64) | 1 | 1 | 1 | 0*63 | V_o(64)]

TRACE = False          # set by test.py to capture a HW profile
LAST_RESULTS = None    # BassKernelResults of the most recent run

_BUILT = None


def _install_trace_shim():
    """The image's antenv lacks axon_hooks; recreate it so trace=True works."""
    import sys, types
    if "antenv.axon_hooks" in sys.modules:
        return
    mod = types.ModuleType("antenv.axon_hooks")
    mod._hook = None
    mod.set_axon_ntff_profile_hook = lambda h: setattr(mod, "_hook", h)
    mod.get_axon_ntff_profile_hook = lambda: mod._hook
    sys.modules["antenv.axon_hooks"] = mod
    import antenv
    antenv.axon_hooks = mod
    from trn_agent_boot.trn_boot import _ntff_profile_via_ctypes
    mod.set_axon_ntff_profile_hook(_ntff_profile_via_ctypes("/opt/axon/libaxon_pjrt.so"))


def _build():
    import concourse.tile as tile
    from concourse import bacc, mybir

    BF = mybir.dt.bfloat16
    F32 = mybir.dt.float32
    AF = mybir.ActivationFunctionType

    nc = bacc.Bacc("TRN2", target_bir_lowering=False, debug=False, num_devices=N_CORES)

    xT_d = nc.dram_tensor("xT", [BPC, C, N], BF, kind="ExternalInput").ap()
    w_d = {
        wn: nc.dram_tensor(wn, [C, C], BF, kind="ExternalInput").ap()
        for wn in ("wq", "wk", "wv", "wvc", "wp")
    }
    bias_d = nc.dram_tensor("bias", [128, C], F32, kind="ExternalInput").ap()
    out_d = nc.dram_tensor("out", [BPC, N, C], F32, kind="ExternalOutput").ap()

    with tile.TileContext(nc) as tc:
        from contextlib import ExitStack
        from concourse import library_config

        with ExitStack() as ctx:
            wpool = ctx.enter_context(tc.tile_pool(name="wpool", bufs=1))
            sb = ctx.enter_context(tc.tile_pool(name="sb", bufs=1))
            ps = ctx.enter_context(tc.tile_pool(name="ps", bufs=1, space="PSUM"))

            # partition_broadcast lives in the gpsimd 'attn' library; the
            # default 'standard' library executes it as garbage on HW
            nc.gpsimd.load_library(library_config.attn)

            # ---- constants: weights + bias ----
            w_sb = {}
            for wn in ("wq", "wk", "wv", "wvc", "wp"):
                tiles = []
                for kc in range(KC):
                    t = wpool.tile([128, C], BF, name=f"{wn}_{kc}", tag=f"{wn}_{kc}")
                    nc.sync.dma_start(t[:], w_d[wn][kc * 128:(kc + 1) * 128, :])
                    tiles.append(t)
                w_sb[wn] = tiles
            bias_sb = wpool.tile([128, C], F32, name="bias_sb", tag="bias_sb")
            nc.sync.dma_start(bias_sb[:], bias_d[:])

            # ---- rotating state shared across the two items ----
            xT = {}     # (it, kc) -> tile

            def load_xT(it):
                for kc in range(KC):
                    t = sb.tile([128, N], BF, name=f"xT_{it}_{kc}", tag="xT", bufs=8)
                    nc.sync.dma_start(t[:], xT_d[it, kc * 128:(kc + 1) * 128, :])
                    xT[(it, kc)] = t

            # ---------- projection helpers ----------
            def qkproj(it, t_, qT, kTh):
                """q and k projections for head pair t_ of item it."""
                dst = sb.tile([128, N], BF, name=f"qT_{it}_{t_}", tag="qT", bufs=3)
                for (qs, qw) in QP:
                    pp = ps.tile([128, 512], F32, name="pp", tag="pp", bufs=2)
                    for kc in range(KC):
                        nc.tensor.matmul(
                            pp[:, 0:qw],
                            lhsT=w_sb["wq"][kc][:, t_ * 128:(t_ + 1) * 128],
                            rhs=xT[(it, kc)][:, qs:qs + qw],
                            start=(kc == 0), stop=(kc == KC - 1),
                        )
                    nc.any.tensor_copy(dst[:, qs:qs + qw], pp[:, 0:qw])
                qT[t_] = dst
                # k^T per head, zero-padded to 128 partitions so S^T runs as a
                # plain K=128 matmul (no PE row tiling -- T8 tile corrupts on HW)
                ke = sb.tile([128, N], BF, name=f"kTh_{it}_{2*t_}", tag="kT", bufs=6)
                ko = sb.tile([128, N], BF, name=f"kTh_{it}_{2*t_+1}", tag="kT", bufs=6)
                nc.vector.memset(ke[64:128, :], 0.0)
                nc.vector.memset(ko[0:64, :], 0.0)
                for (qs, qw) in QP:
                    pp = ps.tile([128, 512], F32, name="pp", tag="pp", bufs=2)
                    for kc in range(KC):
                        nc.tensor.matmul(
                            pp[:, 0:qw],
                            lhsT=w_sb["wk"][kc][:, t_ * 128:(t_ + 1) * 128],
                            rhs=xT[(it, kc)][:, qs:qs + qw],
                            start=(kc == 0), stop=(kc == KC - 1),
                        )
                    nc.any.tensor_copy(ke[0:64, qs:qs + qw], pp[0:64, 0:qw])
                    nc.any.tensor_copy(ko[64:128, qs:qs + qw], pp[64:128, 0:qw])
                kTh[2 * t_] = ke
                kTh[2 * t_ + 1] = ko

            def vproj_group(it, c, wn, tg, dst_map):
                """values for key chunk c of item it, matrix wn (wv/wvc)."""
                ts, tsz = KCH[c]
                dst = sb.tile([128, NPAIR * PW], BF, name=f"{tg}_{it}_{c}",
                              tag=tg, bufs=9)
                if tsz < 128:
                    # stationary loads may read all 128 partitions; keep
                    # the unwritten tail finite
                    nc.vector.memset(dst[:, :], 0.0)
                dvw = dst[0:tsz, :].rearrange("p (g c) -> p g c", c=PW)
                for (cs, cw) in CPASS:
                    pp = ps.tile([128, 512], F32, name="pp", tag="pp", bufs=2)
                    for kc in range(KC):
                        nc.tensor.matmul(
                            pp[0:tsz, 0:cw],
                            lhsT=xT[(it, kc)][:, ts:ts + tsz],
                            rhs=w_sb[wn][kc][:, cs:cs + cw],
                            start=(kc == 0), stop=(kc == KC - 1),
                        )
                    g0, gn = (0, 4) if cs == 0 else (4, 2)
                    src = pp[0:tsz, 0:cw].rearrange("p (g r d) -> p g r d", r=2, d=D)
                    nc.any.tensor_copy(dvw[:, g0:g0 + gn, 0:D], src[:, :, 0, :])
                    nc.any.tensor_copy(dvw[:, g0:g0 + gn, 130:194], src[:, :, 1, :])
                nc.vector.memset(dvw[:, :, 64:67], 1.0)
                nc.vector.memset(dvw[:, :, 67:130], 0.0)
                dst_map[c] = dst

            def make_mixes(it, v_sb, vc_sb):
                # mixed tiles for the key chunk straddling M1 (chunk 4: key 512
                # is modality-a, keys 513.. are modality-v)
                amix = sb.tile([128, NPAIR * PW], BF, name=f"amix_{it}", tag="amix", bufs=2)
                vmix = sb.tile([128, NPAIR * PW], BF, name=f"vmix_{it}", tag="vmix", bufs=2)
                nc.vector.tensor_copy(amix[:, :], vc_sb[4][:, :])
                nc.vector.tensor_copy(amix[0:1, :], v_sb[4][0:1, :])
                nc.vector.tensor_copy(vmix[:, :], v_sb[4][:, :])
                nc.vector.tensor_copy(vmix[0:1, :], vc_sb[4][0:1, :])
                return amix, vmix

            # ---------- attention iteration ----------
            def attn_iter(it, p, par, qT, kTh, v_sb, vc_sb, amix, vmix, ou, den):
                # S^T (keys on partitions) one key chunk at a time, into a
                # 2-bank PSUM tile; one exp per chunk covering all 906 queries
                exps = []
                for c, (ks, ksz) in enumerate(KCH):
                    sc = ps.tile([128, 1024], F32, name="sc", tag="sc", bufs=2)
                    nc.tensor.matmul(sc[0:ksz, 0:512],
                                     lhsT=kTh[2 * p + par][:, ks:ks + ksz],
                                     rhs=qT[p][:, 0:512], start=True, stop=True)
                    nc.tensor.matmul(sc[0:ksz, 512:906],
                                     lhsT=kTh[2 * p + par][:, ks:ks + ksz],
                                     rhs=qT[p][:, 512:906], start=True, stop=True)
                    e = sb.tile([128, 908], BF, name="ee", tag="ee", bufs=9)
                    nc.scalar.activation(e[0:ksz, 0:906], sc[0:ksz, 0:906],
                                         AF.Exp, scale=SCALE)
                    exps.append(e)

                if par == 0:
                    rows = slice(0, 65)
                    csl = slice(p * PW, p * PW + 65)          # [V_even | 1]
                    drow, orows = 64, slice(0, 64)
                else:
                    rows = slice(0, 128)
                    csl = slice(p * PW + 66, p * PW + PW)     # [1 | 0*63 | V_odd]
                    drow, orows = 0, slice(64, 128)

                t1 = ps.tile([128, 512], F32, name="t1", tag="t1", bufs=1)
                t2 = ps.tile([128, 512], F32, name="t2", tag="t2", bufs=1)

                def va(c):
                    return amix if c == 4 else (v_sb[c] if c < 4 else vc_sb[c])

                def vv(c):
                    return vmix if c == 4 else (vc_sb[c] if c < 4 else v_sb[c])

                # modality-a queries q in [0,512)
                for c, (ks, ksz) in enumerate(KCH):
                    nc.tensor.matmul(t1[rows, 0:512], lhsT=va(c)[0:ksz, csl],
                                     rhs=exps[c][0:ksz, 0:512],
                                     start=(c == 0), stop=(c == NCH - 1))
                # modality-v queries q in [512,906) (col 0 of the block, q=512,
                # is modality-a and gets fixed up by the racc column), plus the
                # q=512 a-modality column accumulated into col 400 of the same
                # bank as one extended accumulation group (ordered by the
                # critical section; racc matmuls carry start=False so they
                # accumulate under o2's group clear)
                with tc.tile_critical():
                    for c, (ks, ksz) in enumerate(KCH):
                        nc.tensor.matmul(t2[rows, 0:394], lhsT=vv(c)[0:ksz, csl],
                                         rhs=exps[c][0:ksz, 512:906],
                                         start=(c == 0), stop=False)
                    for c, (ks, ksz) in enumerate(KCH):
                        nc.tensor.matmul(t2[rows, 400:401], lhsT=va(c)[0:ksz, csl],
                                         rhs=exps[c][0:ksz, 512:513],
                                         start=False, stop=(c == NCH - 1))

                # evacuate PSUM immediately (unnormalized, bf16); denominators
                # go to a staging row then DMA into the per-item gather tile
                j = 2 * p + par
                ob = ou[p]
                nc.vector.tensor_copy(ob[orows, 0:512], t1[orows, 0:512])
                nc.vector.tensor_copy(ob[orows, 512:906], t2[orows, 0:394])
                nc.vector.tensor_copy(ob[orows, 512:513], t2[orows, 400:401])
                dstage = sb.tile([128, 908], BF, name="dstage", tag="dstage", bufs=2)
                dr = slice(drow, drow + 1)
                nc.vector.tensor_copy(dstage[dr, 0:512], t1[dr, 0:512])
                nc.vector.tensor_copy(dstage[dr, 513:906], t2[dr, 1:394])
                nc.vector.tensor_copy(dstage[dr, 512:513], t2[dr, 400:401])
                nc.sync.dma_start(den[j:j + 1, 0:906], dstage[dr, 0:906])

            def normalize(it, ou, den):
                # one batched reciprocal for all 12 (pair, parity) rows, then
                # per-row broadcast + in-place multiply
                with nc.allow_low_precision(reason="softmax recip in bf16"):
                    nc.vector.reciprocal(den[0:12, 0:906], den[0:12, 0:906])
                for p in range(NPAIR):
                    for par in range(2):
                        j = 2 * p + par
                        orows = slice(0, 64) if par == 0 else slice(64, 128)
                        stg = sb.tile([1, 908], BF, name="stg", tag="stg", bufs=2)
                        nc.sync.dma_start(stg[0:1, 0:906], den[j:j + 1, 0:906])
                        bc2 = sb.tile([128, 908], BF, name="bc2", tag="bc2", bufs=2)
                        nc.gpsimd.partition_broadcast(bc2[:, 0:906], stg[0:1, 0:906])
                        nc.vector.tensor_mul(ou[p][orows, 0:906], ou[p][orows, 0:906],
                                             bc2[orows, 0:906])

            def outproj_group(it, c, cs_i, ou):
                ts, tsz = KCH[c]
                cs, cw = CPASS[cs_i]
                pp = ps.tile([128, 512], F32, name="pp", tag="pp", bufs=2)
                for kp in range(NPAIR):
                    nc.tensor.matmul(
                        pp[0:tsz, 0:cw],
                        lhsT=ou[kp][:, ts:ts + tsz],
                        rhs=w_sb["wp"][kp][:, cs:cs + cw],
                        start=(kp == 0), stop=(kp == NPAIR - 1),
                    )
                obt = sb.tile([128, 512], F32, name="obt", tag="obt", bufs=2)
                nc.vector.tensor_add(obt[0:tsz, 0:cw], pp[0:tsz, 0:cw],
                                     bias_sb[0:tsz, cs:cs + cw])
                nc.sync.dma_start(out_d[it, ts:ts + tsz, cs:cs + cw], obt[0:tsz, 0:cw])

            # ================= pipeline =================
            state = {}
            for it in range(BPC):
                state[it] = dict(qT={}, kTh={}, v={}, vc={},
                                 ou=[], den=None, amix=None, vmix=None)

            def alloc_item(it):
                s = state[it]
                s["ou"] = [
                    sb.tile([128, 908], BF, name=f"ou_{it}_{p}", tag="ou", bufs=13)
                    for p in range(NPAIR)
                ]
                s["den"] = sb.tile([12, 908], BF, name=f"den_{it}", tag="den", bufs=2)

            # A0: load + values for item 0
            load_xT(0)
            s0 = state[0]
            for c in range(NCH):
                vproj_group(0, c, "wv", "v", s0["v"])
                vproj_group(0, c, "wvc", "vc", s0["vc"])
            s0["amix"], s0["vmix"] = make_mixes(0, s0["v"], s0["vc"])
            alloc_item(0)

            # B phases: attention with q/k lookahead; B0 also streams xT(1),
            # B1 interleaves item0's output projection
            for it in range(BPC):
                s = state[it]
                qkproj(it, 0, s["qT"], s["kTh"])
                op_sched = [2, 2, 2, 2, 1, 1, 1, 1, 1, 1, 1, 1]  # 16 groups
                op_done = 0
                for idx in range(12):
                    p, par = idx // 2, idx % 2
                    if par == 0 and p < NPAIR - 1:
                        qkproj(it, p + 1, s["qT"], s["kTh"])
                    attn_iter(it, p, par, s["qT"], s["kTh"], s["v"], s["vc"],
                              s["amix"], s["vmix"], s["ou"], s["den"])
                    if it == 0 and idx == 8:
                        load_xT(1)
                    if it == 1:
                        for _ in range(op_sched[idx]):
                            c, cs_i = op_done // 2, op_done % 2
                            outproj_group(0, c, cs_i, state[0]["ou"])
                            op_done += 1
                normalize(it, s["ou"], s["den"])
                if it == 0:
                    # A1: values for item 1 (overlaps normalize(0) on PE)
                    s1 = state[1]
                    for c in range(NCH):
                        vproj_group(1, c, "wv", "v", s1["v"])
                        vproj_group(1, c, "wvc", "vc", s1["vc"])
                    s1["amix"], s1["vmix"] = make_mixes(1, s1["v"], s1["vc"])
                    alloc_item(1)

            # C: item 1 output projection
            for c in range(NCH):
                for cs_i in range(2):
                    outproj_group(1, c, cs_i, state[1]["ou"])

    nc.compile()
    return nc


def _get_built():
    global _BUILT
    if _BUILT is None:
        _BUILT = _build()
    return _BUILT


def kernel(x, Wq, Wk, Wv, Wvc, Wp, bp):
    global LAST_RESULTS
    from concourse.bass_utils import run_bass_kernel_spmd

    x = np.asarray(x, dtype=np.float32)
    bf = ml_dtypes.bfloat16
    xT = np.ascontiguousarray(x.transpose(0, 2, 1)).astype(bf)      # (B, C, N)
    ws = {
        "wq": np.asarray(Wq, dtype=np.float32).astype(bf),
        "wk": np.asarray(Wk, dtype=np.float32).astype(bf),
        "wv": np.asarray(Wv, dtype=np.float32).astype(bf),
        "wvc": np.asarray(Wvc, dtype=np.float32).astype(bf),
        "wp": np.asarray(Wp, dtype=np.float32).astype(bf),
    }
    bias = np.ascontiguousarray(
        np.broadcast_to(np.asarray(bp, dtype=np.float32), (128, C))
    )

    if TRACE:
        _install_trace_shim()

    nc = _get_built()
    in_maps = []
    for i in range(N_CORES):
        m = {"xT": np.ascontiguousarray(xT[i * BPC:(i + 1) * BPC]), "bias": bias}
        m.update(ws)
        in_maps.append(m)

    res = run_bass_kernel_spmd(nc, in_maps, list(range(N_CORES)), trace=TRACE,
                               stitch_traces=False)
    LAST_RESULTS = res
    out = np.concatenate([res.results[i]["out"] for i in range(N_CORES)], axis=0)
    return out


# revision 12
# speedup vs baseline: 2.2215x; 1.0038x over previous
"""Multi-modality double-value attention on 8 TRN2 NeuronCores.

Sharding: data-parallel over batch (16 items -> 2 per core). Each core runs
the full attention block for its 2 items; weights are replicated. No
collectives. Host pre-transposes x to x^T and casts inputs to bf16; compute
is bf16 with fp32 PSUM accumulation; output is fp32.

v2 pipeline: the two items are software-pipelined so the PE never idles long
enough for the HAM clock gate to re-throttle. Scores for one (pair, parity)
go into a single 2-bank PSUM tile and are exponentiated with one 906-col
ACT instruction; attention outputs are evacuated to SBUF immediately
(unnormalized, bf16) so PSUM banks recycle fast; softmax division happens
late via one batched reciprocal per item + gpsimd row-broadcasts.
"""

import numpy as np
import ml_dtypes

B, N, C = 16, 906, 768
H = 12
D = 64
M1 = 513
N_CORES = 8
BPC = B // N_CORES          # batch items per core
KC = C // 128               # 6 contraction chunks over C
NPAIR = H // 2              # 6 head pairs
NCH = (N + 127) // 128      # 8 key/token chunks over N
KCH = [(i * 128, min(128, N - i * 128)) for i in range(NCH)]
QP = [(0, 512), (512, N - 512)]      # column passes over N
CPASS = [(0, 512), (512, C - 512)]   # column passes over C
SCALE = D ** -0.5
PW = 194  # per-head-pair value block: [V_e(64) | 1 | 1 | 1 | 0*63 | V_o(64)]

TRACE = False          # set by test.py to capture a HW profile
LAST_RESULTS = None    # BassKernelResults of the most recent run

_BUILT = None


def _install_trace_shim():
    """The image's antenv lacks axon_hooks; recreate it so trace=True works."""
    import sys, types
    if "antenv.axon_hooks" in sys.modules:
        return
    mod = types.ModuleType("antenv.axon_hooks")
    mod._hook = None
    mod.set_axon_ntff_profile_hook = lambda h: setattr(mod, "_hook", h)
    mod.get_axon_ntff_profile_hook = lambda: mod._hook
    sys.modules["antenv.axon_hooks"] = mod
    import antenv
    antenv.axon_hooks = mod
    from trn_agent_boot.trn_boot import _ntff_profile_via_ctypes
    mod.set_axon_ntff_profile_hook(_ntff_profile_via_ctypes("/opt/axon/libaxon_pjrt.so"))


def _build():
    import concourse.tile as tile
    from concourse import bacc, mybir

    BF = mybir.dt.bfloat16
    F32 = mybir.dt.float32
    AF = mybir.ActivationFunctionType

    nc = bacc.Bacc("TRN2", target_bir_lowering=False, debug=False, num_devices=N_CORES)

    xT_d = nc.dram_tensor("xT", [BPC, C, N], BF, kind="ExternalInput").ap()
    w_d = {
        wn: nc.dram_tensor(wn, [C, C], BF, kind="ExternalInput").ap()
        for wn in ("wq", "wk", "wv", "wvc", "wp")
    }
    bias_d = nc.dram_tensor("bias", [128, C], F32, kind="ExternalInput").ap()
    out_d = nc.dram_tensor("out", [BPC, N, C], F32, kind="ExternalOutput").ap()

    with tile.TileContext(nc) as tc:
        from contextlib import ExitStack
        from concourse import library_config

        with ExitStack() as ctx:
            wpool = ctx.enter_context(tc.tile_pool(name="wpool", bufs=1))
            sb = ctx.enter_context(tc.tile_pool(name="sb", bufs=1))
            ps = ctx.enter_context(tc.tile_pool(name="ps", bufs=1, space="PSUM"))

            # partition_broadcast lives in the gpsimd 'attn' library; the
            # default 'standard' library executes it as garbage on HW
            nc.gpsimd.load_library(library_config.attn)

            # ---- constants: weights + bias ----
            # DMA order matters for the pipeline head: the first compute
            # phase (vproj of item 0) needs wv/wvc, so load those first
            w_sb = {}

            def load_weights(names):
                for wn in names:
                    tiles = []
                    for kc in range(KC):
                        t = wpool.tile([128, C], BF, name=f"{wn}_{kc}", tag=f"{wn}_{kc}")
                        nc.sync.dma_start(t[:], w_d[wn][kc * 128:(kc + 1) * 128, :])
                        tiles.append(t)
                    w_sb[wn] = tiles

            load_weights(("wv", "wvc"))
            bias_sb = wpool.tile([128, C], F32, name="bias_sb", tag="bias_sb")

            # ---- rotating state shared across the two items ----
            xT = {}     # (it, kc) -> tile

            def load_xT(it):
                for kc in range(KC):
                    t = sb.tile([128, N], BF, name=f"xT_{it}_{kc}", tag="xT", bufs=8)
                    nc.sync.dma_start(t[:], xT_d[it, kc * 128:(kc + 1) * 128, :])
                    xT[(it, kc)] = t

            # ---------- projection helpers ----------
            def qkproj(it, t_, qT, kTh):
                """q and k projections for head pair t_ of item it."""
                dst = sb.tile([128, N], BF, name=f"qT_{it}_{t_}", tag="qT", bufs=3)
                for (qs, qw) in QP:
                    pp = ps.tile([128, 512], F32, name="pp", tag="pp", bufs=2)
                    for kc in range(KC):
                        nc.tensor.matmul(
                            pp[:, 0:qw],
                            lhsT=w_sb["wq"][kc][:, t_ * 128:(t_ + 1) * 128],
                            rhs=xT[(it, kc)][:, qs:qs + qw],
                            start=(kc == 0), stop=(kc == KC - 1),
                        )
                    # explicit DVE: ACT is saturated by exp during B phases
                    nc.vector.tensor_copy(dst[:, qs:qs + qw], pp[:, 0:qw])
                qT[t_] = dst
                # k^T per head, zero-padded to 128 partitions so S^T runs as a
                # plain K=128 matmul (no PE row tiling -- T8 tile corrupts on HW)
                ke = sb.tile([128, N], BF, name=f"kTh_{it}_{2*t_}", tag="kT", bufs=6)
                ko = sb.tile([128, N], BF, name=f"kTh_{it}_{2*t_+1}", tag="kT", bufs=6)
                nc.vector.memset(ke[64:128, :], 0.0)
                nc.vector.memset(ko[0:64, :], 0.0)
                for (qs, qw) in QP:
                    pp = ps.tile([128, 512], F32, name="pp", tag="pp", bufs=2)
                    for kc in range(KC):
                        nc.tensor.matmul(
                            pp[:, 0:qw],
                            lhsT=w_sb["wk"][kc][:, t_ * 128:(t_ + 1) * 128],
                            rhs=xT[(it, kc)][:, qs:qs + qw],
                            start=(kc == 0), stop=(kc == KC - 1),
                        )
                    nc.vector.tensor_copy(ke[0:64, qs:qs + qw], pp[0:64, 0:qw])
                    nc.vector.tensor_copy(ko[64:128, qs:qs + qw], pp[64:128, 0:qw])
                kTh[2 * t_] = ke
                kTh[2 * t_ + 1] = ko

            def vproj_group(it, c, wn, tg, dst_map):
                """values for key chunk c of item it, matrix wn (wv/wvc)."""
                ts, tsz = KCH[c]
                dst = sb.tile([128, NPAIR * PW], BF, name=f"{tg}_{it}_{c}",
                              tag=tg, bufs=9)
                if tsz < 128:
                    # stationary loads may read all 128 partitions; keep
                    # the unwritten tail finite
                    nc.vector.memset(dst[:, :], 0.0)
                dvw = dst[0:tsz, :].rearrange("p (g c) -> p g c", c=PW)
                for (cs, cw) in CPASS:
                    pp = ps.tile([128, 512], F32, name="pp", tag="pp", bufs=2)
                    for kc in range(KC):
                        nc.tensor.matmul(
                            pp[0:tsz, 0:cw],
                            lhsT=xT[(it, kc)][:, ts:ts + tsz],
                            rhs=w_sb[wn][kc][:, cs:cs + cw],
                            start=(kc == 0), stop=(kc == KC - 1),
                        )
                    g0, gn = (0, 4) if cs == 0 else (4, 2)
                    src = pp[0:tsz, 0:cw].rearrange("p (g r d) -> p g r d", r=2, d=D)
                    nc.any.tensor_copy(dvw[:, g0:g0 + gn, 0:D], src[:, :, 0, :])
                    nc.any.tensor_copy(dvw[:, g0:g0 + gn, 130:194], src[:, :, 1, :])
                nc.vector.memset(dvw[:, :, 64:67], 1.0)
                nc.vector.memset(dvw[:, :, 67:130], 0.0)
                dst_map[c] = dst

            def make_mixes(it, v_sb, vc_sb):
                # mixed tiles for the key chunk straddling M1 (chunk 4: key 512
                # is modality-a, keys 513.. are modality-v)
                amix = sb.tile([128, NPAIR * PW], BF, name=f"amix_{it}", tag="amix", bufs=2)
                vmix = sb.tile([128, NPAIR * PW], BF, name=f"vmix_{it}", tag="vmix", bufs=2)
                nc.vector.tensor_copy(amix[:, :], vc_sb[4][:, :])
                nc.vector.tensor_copy(amix[0:1, :], v_sb[4][0:1, :])
                nc.vector.tensor_copy(vmix[:, :], v_sb[4][:, :])
                nc.vector.tensor_copy(vmix[0:1, :], vc_sb[4][0:1, :])
                return amix, vmix

            # ---------- attention iteration ----------
            def attn_iter(it, p, par, qT, kTh, v_sb, vc_sb, amix, vmix, ou, den):
                # S^T (keys on partitions) one key chunk at a time, into a
                # 2-bank PSUM tile; one exp per chunk covering all 906 queries
                exps = []
                for c, (ks, ksz) in enumerate(KCH):
                    sc = ps.tile([128, 1024], F32, name="sc", tag="sc", bufs=2)
                    nc.tensor.matmul(sc[0:ksz, 0:512],
                                     lhsT=kTh[2 * p + par][:, ks:ks + ksz],
                                     rhs=qT[p][:, 0:512], start=True, stop=True)
                    nc.tensor.matmul(sc[0:ksz, 512:906],
                                     lhsT=kTh[2 * p + par][:, ks:ks + ksz],
                                     rhs=qT[p][:, 512:906], start=True, stop=True)
                    e = sb.tile([128, 908], BF, name="ee", tag="ee", bufs=9)
                    nc.scalar.activation(e[0:ksz, 0:906], sc[0:ksz, 0:906],
                                         AF.Exp, scale=SCALE)
                    exps.append(e)

                if par == 0:
                    rows = slice(0, 65)
                    csl = slice(p * PW, p * PW + 65)          # [V_even | 1]
                    drow, orows = 64, slice(0, 64)
                else:
                    rows = slice(0, 128)
                    csl = slice(p * PW + 66, p * PW + PW)     # [1 | 0*63 | V_odd]
                    drow, orows = 0, slice(64, 128)

                t1 = ps.tile([128, 512], F32, name="t1", tag="t1", bufs=1)
                t2 = ps.tile([128, 512], F32, name="t2", tag="t2", bufs=1)

                def va(c):
                    return amix if c == 4 else (v_sb[c] if c < 4 else vc_sb[c])

                def vv(c):
                    return vmix if c == 4 else (vc_sb[c] if c < 4 else v_sb[c])

                # modality-a queries q in [0,512)
                for c, (ks, ksz) in enumerate(KCH):
                    nc.tensor.matmul(t1[rows, 0:512], lhsT=va(c)[0:ksz, csl],
                                     rhs=exps[c][0:ksz, 0:512],
                                     start=(c == 0), stop=(c == NCH - 1))
                # modality-v queries q in [512,906) (col 0 of the block, q=512,
                # is modality-a and gets fixed up by the racc column), plus the
                # q=512 a-modality column accumulated into col 400 of the same
                # bank as one extended accumulation group (ordered by the
                # critical section; racc matmuls carry start=False so they
                # accumulate under o2's group clear)
                with tc.tile_critical():
                    for c, (ks, ksz) in enumerate(KCH):
                        nc.tensor.matmul(t2[rows, 0:394], lhsT=vv(c)[0:ksz, csl],
                                         rhs=exps[c][0:ksz, 512:906],
                                         start=(c == 0), stop=False)
                    for c, (ks, ksz) in enumerate(KCH):
                        nc.tensor.matmul(t2[rows, 400:401], lhsT=va(c)[0:ksz, csl],
                                         rhs=exps[c][0:ksz, 512:513],
                                         start=False, stop=(c == NCH - 1))

                # evacuate PSUM immediately (unnormalized, bf16); denominators
                # go to a staging row then DMA into the per-item gather tile
                j = 2 * p + par
                ob = ou[p]
                nc.vector.tensor_copy(ob[orows, 0:512], t1[orows, 0:512])
                nc.vector.tensor_copy(ob[orows, 512:906], t2[orows, 0:394])
                nc.vector.tensor_copy(ob[orows, 512:513], t2[orows, 400:401])
                dstage = sb.tile([128, 908], BF, name="dstage", tag="dstage", bufs=2)
                dr = slice(drow, drow + 1)
                nc.vector.tensor_copy(dstage[dr, 0:512], t1[dr, 0:512])
                nc.vector.tensor_copy(dstage[dr, 513:906], t2[dr, 1:394])
                nc.vector.tensor_copy(dstage[dr, 512:513], t2[dr, 400:401])
                dh = den[j // 6]
                r6 = j % 6
                nc.sync.dma_start(dh[r6:r6 + 1, 0:906], dstage[dr, 0:906])

            def normalize_half(it, ou, den, plo, phi):
                # batched reciprocal for pairs [plo, phi), then per-row
                # broadcast + in-place multiply; called per half-item so the
                # chain overlaps the remaining attention iterations
                dh = den[(2 * plo) // 6]
                with nc.allow_low_precision(reason="softmax recip in bf16"):
                    nc.vector.reciprocal(dh[0:6, 0:906], dh[0:6, 0:906])
                for p in range(plo, phi):
                    for par in range(2):
                        j = 2 * p + par
                        r6 = j % 6
                        orows = slice(0, 64) if par == 0 else slice(64, 128)
                        stg = sb.tile([1, 908], BF, name="stg", tag="stg", bufs=2)
                        nc.sync.dma_start(stg[0:1, 0:906], dh[r6:r6 + 1, 0:906])
                        bc2 = sb.tile([128, 908], BF, name="bc2", tag="bc2", bufs=2)
                        nc.gpsimd.partition_broadcast(bc2[:, 0:906], stg[0:1, 0:906])
                        nc.vector.tensor_mul(ou[p][orows, 0:906], ou[p][orows, 0:906],
                                             bc2[orows, 0:906])

            def outproj_group(it, c, cs_i, ou):
                ts, tsz = KCH[c]
                cs, cw = CPASS[cs_i]
                pp = ps.tile([128, 512], F32, name="pp", tag="pp", bufs=2)
                for kp in range(NPAIR):
                    nc.tensor.matmul(
                        pp[0:tsz, 0:cw],
                        lhsT=ou[kp][:, ts:ts + tsz],
                        rhs=w_sb["wp"][kp][:, cs:cs + cw],
                        start=(kp == 0), stop=(kp == NPAIR - 1),
                    )
                obt = sb.tile([128, 512], F32, name="obt", tag="obt", bufs=2)
                nc.vector.tensor_add(obt[0:tsz, 0:cw], pp[0:tsz, 0:cw],
                                     bias_sb[0:tsz, cs:cs + cw])
                nc.sync.dma_start(out_d[it, ts:ts + tsz, cs:cs + cw], obt[0:tsz, 0:cw])

            def outproj_wide(it, c, ou):
                # tail variant: both column passes into one 2-bank PSUM tile
                # (the sc tag is free once attention is done), single bias add
                # and single full-row DMA -- fewer serialization points
                ts, tsz = KCH[c]
                pw = ps.tile([128, 1024], F32, name="sc", tag="sc", bufs=2)
                for cs_i, (cs, cw) in enumerate(CPASS):
                    for kp in range(NPAIR):
                        nc.tensor.matmul(
                            pw[0:tsz, cs:cs + cw],
                            lhsT=ou[kp][:, ts:ts + tsz],
                            rhs=w_sb["wp"][kp][:, cs:cs + cw],
                            start=(kp == 0), stop=(kp == NPAIR - 1),
                        )
                obw = sb.tile([128, 768], F32, name="obw", tag="obw", bufs=3)
                nc.vector.tensor_add(obw[0:tsz, 0:768], pw[0:tsz, 0:768],
                                     bias_sb[0:tsz, 0:768])
                nc.sync.dma_start(out_d[it, ts:ts + tsz, 0:768], obw[0:tsz, 0:768])

            # ================= pipeline =================
            state = {}
            for it in range(BPC):
                state[it] = dict(qT={}, kTh={}, v={}, vc={},
                                 ou=[], den=None, amix=None, vmix=None)

            def alloc_item(it):
                s = state[it]
                s["ou"] = [
                    sb.tile([128, 908], BF, name=f"ou_{it}_{p}", tag="ou", bufs=13)
                    for p in range(NPAIR)
                ]
                s["den"] = [
                    sb.tile([6, 908], BF, name=f"den_{it}_{h}", tag="den", bufs=4)
                    for h in range(2)
                ]

            # A0: load + values for item 0 (xT and q/k/p weights DMA after
            # the wv/wvc weights so value projection can start early)
            load_xT(0)
            load_weights(("wq", "wk", "wp"))
            nc.sync.dma_start(bias_sb[:], bias_d[:])
            s0 = state[0]
            for c in range(NCH):
                vproj_group(0, c, "wv", "v", s0["v"])
                vproj_group(0, c, "wvc", "vc", s0["vc"])
            s0["amix"], s0["vmix"] = make_mixes(0, s0["v"], s0["vc"])
            alloc_item(0)

            # B phases: attention with q/k lookahead; B0 also streams xT(1),
            # B1 interleaves item0's output projection
            for it in range(BPC):
                s = state[it]
                qkproj(it, 0, s["qT"], s["kTh"])
                op_sched = [2, 2, 2, 2, 1, 1, 1, 1, 1, 1, 1, 1]  # 16 groups
                op_done = 0
                for idx in range(12):
                    p, par = idx // 2, idx % 2
                    if par == 0 and p < NPAIR - 1:
                        qkproj(it, p + 1, s["qT"], s["kTh"])
                    attn_iter(it, p, par, s["qT"], s["kTh"], s["v"], s["vc"],
                              s["amix"], s["vmix"], s["ou"], s["den"])
                    if idx == 5:
                        # first-half normalize overlaps the remaining pairs
                        normalize_half(it, s["ou"], s["den"], 0, 3)
                    if it == 0 and idx == 6:
                        load_xT(1)
                    if it == 1:
                        for _ in range(op_sched[idx]):
                            c, cs_i = op_done // 2, op_done % 2
                            outproj_group(0, c, cs_i, state[0]["ou"])
                            op_done += 1
                normalize_half(it, s["ou"], s["den"], 3, 6)
                if it == 0:
                    # A1: values for item 1 (overlaps normalize(0) on PE)
                    s1 = state[1]
                    for c in range(NCH):
                        vproj_group(1, c, "wv", "v", s1["v"])
                        vproj_group(1, c, "wvc", "vc", s1["vc"])
                    s1["amix"], s1["vmix"] = make_mixes(1, s1["v"], s1["vc"])
                    alloc_item(1)

            # C: item 1 output projection (wide tail groups)
            for c in range(NCH):
                outproj_wide(1, c, state[1]["ou"])

    nc.compile()
    return nc


def _get_built():
    global _BUILT
    if _BUILT is None:
        _BUILT = _build()
    return _BUILT


def kernel(x, Wq, Wk, Wv, Wvc, Wp, bp):
    global LAST_RESULTS
    from concourse.bass_utils import run_bass_kernel_spmd

    x = np.asarray(x, dtype=np.float32)
    bf = ml_dtypes.bfloat16
    xT = np.ascontiguousarray(x.transpose(0, 2, 1)).astype(bf)      # (B, C, N)
    ws = {
        "wq": np.asarray(Wq, dtype=np.float32).astype(bf),
        "wk": np.asarray(Wk, dtype=np.float32).astype(bf),
        "wv": np.asarray(Wv, dtype=np.float32).astype(bf),
        "wvc": np.asarray(Wvc, dtype=np.float32).astype(bf),
        "wp": np.asarray(Wp, dtype=np.float32).astype(bf),
    }
    bias = np.ascontiguousarray(
        np.broadcast_to(np.asarray(bp, dtype=np.float32), (128, C))
    )

    if TRACE:
        _install_trace_shim()

    nc = _get_built()
    in_maps = []
    for i in range(N_CORES):
        m = {"xT": np.ascontiguousarray(xT[i * BPC:(i + 1) * BPC]), "bias": bias}
        m.update(ws)
        in_maps.append(m)

    res = run_bass_kernel_spmd(nc, in_maps, list(range(N_CORES)), trace=TRACE,
                               stitch_traces=False)
    LAST_RESULTS = res
    out = np.concatenate([res.results[i]["out"] for i in range(N_CORES)], axis=0)
    return out


# revision 13
# speedup vs baseline: 2.2959x; 1.0335x over previous
"""Multi-modality double-value attention on 8 TRN2 NeuronCores.

Sharding: data-parallel over batch (16 items -> 2 per core). Each core runs
the full attention block for its 2 items; weights are replicated. No
collectives. Host pre-transposes x to x^T and casts inputs to bf16; compute
is bf16 with fp32 PSUM accumulation; output is fp32.

v2 pipeline: the two items are software-pipelined so the PE never idles long
enough for the HAM clock gate to re-throttle. Scores for one (pair, parity)
go into a single 2-bank PSUM tile and are exponentiated with one 906-col
ACT instruction; attention outputs are evacuated to SBUF immediately
(unnormalized, bf16) so PSUM banks recycle fast; softmax division happens
late via one batched reciprocal per item + gpsimd row-broadcasts.
"""

import numpy as np
import ml_dtypes

B, N, C = 16, 906, 768
H = 12
D = 64
M1 = 513
N_CORES = 8
BPC = B // N_CORES          # batch items per core
KC = C // 128               # 6 contraction chunks over C
NPAIR = H // 2              # 6 head pairs
NCH = (N + 127) // 128      # 8 key/token chunks over N
KCH = [(i * 128, min(128, N - i * 128)) for i in range(NCH)]
QP = [(0, 512), (512, N - 512)]      # column passes over N
CPASS = [(0, 512), (512, C - 512)]   # column passes over C
SCALE = D ** -0.5
PW = 194  # per-head-pair value block: [V_e(64) | 1 | 1 | 1 | 0*63 | V_o(64)]

TRACE = False          # set by test.py to capture a HW profile
LAST_RESULTS = None    # BassKernelResults of the most recent run

_BUILT = None


def _install_trace_shim():
    """The image's antenv lacks axon_hooks; recreate it so trace=True works."""
    import sys, types
    if "antenv.axon_hooks" in sys.modules:
        return
    mod = types.ModuleType("antenv.axon_hooks")
    mod._hook = None
    mod.set_axon_ntff_profile_hook = lambda h: setattr(mod, "_hook", h)
    mod.get_axon_ntff_profile_hook = lambda: mod._hook
    sys.modules["antenv.axon_hooks"] = mod
    import antenv
    antenv.axon_hooks = mod
    from trn_agent_boot.trn_boot import _ntff_profile_via_ctypes
    mod.set_axon_ntff_profile_hook(_ntff_profile_via_ctypes("/opt/axon/libaxon_pjrt.so"))


def _build():
    import concourse.tile as tile
    from concourse import bacc, mybir

    BF = mybir.dt.bfloat16
    F32 = mybir.dt.float32
    AF = mybir.ActivationFunctionType

    nc = bacc.Bacc("TRN2", target_bir_lowering=False, debug=False, num_devices=N_CORES)

    xT_d = nc.dram_tensor("xT", [BPC, C, N], BF, kind="ExternalInput").ap()
    w_d = {
        wn: nc.dram_tensor(wn, [C, C], BF, kind="ExternalInput").ap()
        for wn in ("wq", "wk", "wv", "wvc", "wp")
    }
    bias_d = nc.dram_tensor("bias", [128, C], F32, kind="ExternalInput").ap()
    out_d = nc.dram_tensor("out", [BPC, N, C], F32, kind="ExternalOutput").ap()

    with tile.TileContext(nc) as tc:
        from contextlib import ExitStack
        from concourse import library_config

        with ExitStack() as ctx:
            wpool = ctx.enter_context(tc.tile_pool(name="wpool", bufs=1))
            sb = ctx.enter_context(tc.tile_pool(name="sb", bufs=1))
            ps = ctx.enter_context(tc.tile_pool(name="ps", bufs=1, space="PSUM"))

            # partition_broadcast lives in the gpsimd 'attn' library; the
            # default 'standard' library executes it as garbage on HW
            nc.gpsimd.load_library(library_config.attn)

            # ---- constants: weights + bias ----
            # DMA order matters for the pipeline head: the first compute
            # phase (vproj of item 0) needs wv/wvc, so load those first
            w_sb = {}

            def load_weights(names):
                for wn in names:
                    tiles = []
                    for kc in range(KC):
                        t = wpool.tile([128, C], BF, name=f"{wn}_{kc}", tag=f"{wn}_{kc}")
                        nc.sync.dma_start(t[:], w_d[wn][kc * 128:(kc + 1) * 128, :])
                        tiles.append(t)
                    w_sb[wn] = tiles

            load_weights(("wv",))
            bias_sb = wpool.tile([128, C], F32, name="bias_sb", tag="bias_sb")

            # ---- rotating state shared across the two items ----
            xT = {}     # (it, kc) -> tile

            def load_xT(it):
                for kc in range(KC):
                    t = sb.tile([128, N], BF, name=f"xT_{it}_{kc}", tag="xT", bufs=8)
                    nc.sync.dma_start(t[:], xT_d[it, kc * 128:(kc + 1) * 128, :])
                    xT[(it, kc)] = t

            # ---------- projection helpers ----------
            def qkproj(it, t_, qT, kTh):
                """q and k projections for head pair t_ of item it."""
                dst = sb.tile([128, N], BF, name=f"qT_{it}_{t_}", tag="qT", bufs=4)
                for (qs, qw) in QP:
                    pp = ps.tile([128, 512], F32, name="pp", tag="pp", bufs=2)
                    for kc in range(KC):
                        nc.tensor.matmul(
                            pp[:, 0:qw],
                            lhsT=w_sb["wq"][kc][:, t_ * 128:(t_ + 1) * 128],
                            rhs=xT[(it, kc)][:, qs:qs + qw],
                            start=(kc == 0), stop=(kc == KC - 1),
                        )
                    # explicit DVE: ACT is saturated by exp during B phases
                    nc.vector.tensor_copy(dst[:, qs:qs + qw], pp[:, 0:qw])
                qT[t_] = dst
                # k^T per head, zero-padded to 128 partitions so S^T runs as a
                # plain K=128 matmul (no PE row tiling -- T8 tile corrupts on HW)
                ke = sb.tile([128, N], BF, name=f"kTh_{it}_{2*t_}", tag="kT", bufs=8)
                ko = sb.tile([128, N], BF, name=f"kTh_{it}_{2*t_+1}", tag="kT", bufs=8)
                nc.vector.memset(ke[64:128, :], 0.0)
                nc.vector.memset(ko[0:64, :], 0.0)
                for (qs, qw) in QP:
                    pp = ps.tile([128, 512], F32, name="pp", tag="pp", bufs=2)
                    for kc in range(KC):
                        nc.tensor.matmul(
                            pp[:, 0:qw],
                            lhsT=w_sb["wk"][kc][:, t_ * 128:(t_ + 1) * 128],
                            rhs=xT[(it, kc)][:, qs:qs + qw],
                            start=(kc == 0), stop=(kc == KC - 1),
                        )
                    nc.vector.tensor_copy(ke[0:64, qs:qs + qw], pp[0:64, 0:qw])
                    nc.vector.tensor_copy(ko[64:128, qs:qs + qw], pp[64:128, 0:qw])
                kTh[2 * t_] = ke
                kTh[2 * t_ + 1] = ko

            def vproj_group(it, c, wn, tg, dst_map):
                """values for key chunk c of item it, matrix wn (wv/wvc)."""
                ts, tsz = KCH[c]
                dst = sb.tile([128, NPAIR * PW], BF, name=f"{tg}_{it}_{c}",
                              tag=tg, bufs=9)
                if tsz < 128:
                    # stationary loads may read all 128 partitions; keep
                    # the unwritten tail finite
                    nc.vector.memset(dst[:, :], 0.0)
                dvw = dst[0:tsz, :].rearrange("p (g c) -> p g c", c=PW)
                for (cs, cw) in CPASS:
                    pp = ps.tile([128, 512], F32, name="pp", tag="pp", bufs=2)
                    for kc in range(KC):
                        nc.tensor.matmul(
                            pp[0:tsz, 0:cw],
                            lhsT=xT[(it, kc)][:, ts:ts + tsz],
                            rhs=w_sb[wn][kc][:, cs:cs + cw],
                            start=(kc == 0), stop=(kc == KC - 1),
                        )
                    g0, gn = (0, 4) if cs == 0 else (4, 2)
                    src = pp[0:tsz, 0:cw].rearrange("p (g r d) -> p g r d", r=2, d=D)
                    nc.any.tensor_copy(dvw[:, g0:g0 + gn, 0:D], src[:, :, 0, :])
                    nc.any.tensor_copy(dvw[:, g0:g0 + gn, 130:194], src[:, :, 1, :])
                nc.vector.memset(dvw[:, :, 64:67], 1.0)
                nc.vector.memset(dvw[:, :, 67:130], 0.0)
                dst_map[c] = dst

            def make_mixes(it, v_sb, vc_sb):
                # mixed tiles for the key chunk straddling M1 (chunk 4: key 512
                # is modality-a, keys 513.. are modality-v)
                amix = sb.tile([128, NPAIR * PW], BF, name=f"amix_{it}", tag="amix", bufs=2)
                vmix = sb.tile([128, NPAIR * PW], BF, name=f"vmix_{it}", tag="vmix", bufs=2)
                nc.vector.tensor_copy(amix[:, :], vc_sb[4][:, :])
                nc.vector.tensor_copy(amix[0:1, :], v_sb[4][0:1, :])
                nc.vector.tensor_copy(vmix[:, :], v_sb[4][:, :])
                nc.vector.tensor_copy(vmix[0:1, :], vc_sb[4][0:1, :])
                return amix, vmix

            # ---------- attention iteration ----------
            def attn_iter(it, p, par, qT, kTh, v_sb, vc_sb, amix, vmix, ou, den):
                # S^T (keys on partitions) one key chunk at a time, into a
                # 2-bank PSUM tile; one exp per chunk covering all 906 queries
                exps = []
                for c, (ks, ksz) in enumerate(KCH):
                    sc = ps.tile([128, 1024], F32, name="sc", tag="sc", bufs=2)
                    nc.tensor.matmul(sc[0:ksz, 0:512],
                                     lhsT=kTh[2 * p + par][:, ks:ks + ksz],
                                     rhs=qT[p][:, 0:512], start=True, stop=True)
                    nc.tensor.matmul(sc[0:ksz, 512:906],
                                     lhsT=kTh[2 * p + par][:, ks:ks + ksz],
                                     rhs=qT[p][:, 512:906], start=True, stop=True)
                    e = sb.tile([128, 908], BF, name="ee", tag="ee", bufs=10)
                    nc.scalar.activation(e[0:ksz, 0:906], sc[0:ksz, 0:906],
                                         AF.Exp, scale=SCALE)
                    exps.append(e)

                if par == 0:
                    rows = slice(0, 65)
                    csl = slice(p * PW, p * PW + 65)          # [V_even | 1]
                    drow, orows = 64, slice(0, 64)
                else:
                    rows = slice(0, 128)
                    csl = slice(p * PW + 66, p * PW + PW)     # [1 | 0*63 | V_odd]
                    drow, orows = 0, slice(64, 128)

                t1 = ps.tile([128, 512], F32, name="t1", tag="t1", bufs=1)
                t2 = ps.tile([128, 512], F32, name="t2", tag="t2", bufs=1)

                def va(c):
                    return amix if c == 4 else (v_sb[c] if c < 4 else vc_sb[c])

                def vv(c):
                    return vmix if c == 4 else (vc_sb[c] if c < 4 else v_sb[c])

                # modality-a queries q in [0,512)
                for c, (ks, ksz) in enumerate(KCH):
                    nc.tensor.matmul(t1[rows, 0:512], lhsT=va(c)[0:ksz, csl],
                                     rhs=exps[c][0:ksz, 0:512],
                                     start=(c == 0), stop=(c == NCH - 1))
                # modality-v queries q in [512,906) (col 0 of the block, q=512,
                # is modality-a and gets fixed up by the racc column), plus the
                # q=512 a-modality column accumulated into col 400 of the same
                # bank as one extended accumulation group (ordered by the
                # critical section; racc matmuls carry start=False so they
                # accumulate under o2's group clear)
                with tc.tile_critical():
                    for c, (ks, ksz) in enumerate(KCH):
                        nc.tensor.matmul(t2[rows, 0:394], lhsT=vv(c)[0:ksz, csl],
                                         rhs=exps[c][0:ksz, 512:906],
                                         start=(c == 0), stop=False)
                    for c, (ks, ksz) in enumerate(KCH):
                        nc.tensor.matmul(t2[rows, 400:401], lhsT=va(c)[0:ksz, csl],
                                         rhs=exps[c][0:ksz, 512:513],
                                         start=False, stop=(c == NCH - 1))

                # evacuate PSUM immediately (unnormalized, bf16); denominators
                # go to a staging row then DMA into the per-item gather tile
                j = 2 * p + par
                ob = ou[p]
                nc.vector.tensor_copy(ob[orows, 0:512], t1[orows, 0:512])
                nc.vector.tensor_copy(ob[orows, 512:906], t2[orows, 0:394])
                nc.vector.tensor_copy(ob[orows, 512:513], t2[orows, 400:401])
                dstage = sb.tile([128, 908], BF, name="dstage", tag="dstage", bufs=2)
                dr = slice(drow, drow + 1)
                nc.vector.tensor_copy(dstage[dr, 0:512], t1[dr, 0:512])
                nc.vector.tensor_copy(dstage[dr, 513:906], t2[dr, 1:394])
                nc.vector.tensor_copy(dstage[dr, 512:513], t2[dr, 400:401])
                dh = den[j // 6]
                r6 = j % 6
                nc.sync.dma_start(dh[r6:r6 + 1, 0:906], dstage[dr, 0:906])

            def normalize_half(it, ou, den, plo, phi):
                # batched reciprocal for pairs [plo, phi), then per-row
                # broadcast + in-place multiply; called per half-item so the
                # chain overlaps the remaining attention iterations
                dh = den[(2 * plo) // 6]
                with nc.allow_low_precision(reason="softmax recip in bf16"):
                    nc.vector.reciprocal(dh[0:6, 0:906], dh[0:6, 0:906])
                for p in range(plo, phi):
                    for par in range(2):
                        j = 2 * p + par
                        r6 = j % 6
                        orows = slice(0, 64) if par == 0 else slice(64, 128)
                        stg = sb.tile([1, 908], BF, name="stg", tag="stg", bufs=2)
                        nc.sync.dma_start(stg[0:1, 0:906], dh[r6:r6 + 1, 0:906])
                        bc2 = sb.tile([128, 908], BF, name="bc2", tag="bc2", bufs=2)
                        nc.gpsimd.partition_broadcast(bc2[:, 0:906], stg[0:1, 0:906])
                        nc.vector.tensor_mul(ou[p][orows, 0:906], ou[p][orows, 0:906],
                                             bc2[orows, 0:906])

            def outproj_group(it, c, cs_i, ou):
                ts, tsz = KCH[c]
                cs, cw = CPASS[cs_i]
                pp = ps.tile([128, 512], F32, name="pp", tag="pp", bufs=2)
                for kp in range(NPAIR):
                    nc.tensor.matmul(
                        pp[0:tsz, 0:cw],
                        lhsT=ou[kp][:, ts:ts + tsz],
                        rhs=w_sb["wp"][kp][:, cs:cs + cw],
                        start=(kp == 0), stop=(kp == NPAIR - 1),
                    )
                obt = sb.tile([128, 512], F32, name="obt", tag="obt", bufs=2)
                nc.vector.tensor_add(obt[0:tsz, 0:cw], pp[0:tsz, 0:cw],
                                     bias_sb[0:tsz, cs:cs + cw])
                nc.sync.dma_start(out_d[it, ts:ts + tsz, cs:cs + cw], obt[0:tsz, 0:cw])

            def outproj_wide(it, c, ou):
                # tail variant: both column passes into one 2-bank PSUM tile
                # (the sc tag is free once attention is done), single bias add
                # and single full-row DMA -- fewer serialization points
                ts, tsz = KCH[c]
                pw = ps.tile([128, 1024], F32, name="sc", tag="sc", bufs=2)
                for cs_i, (cs, cw) in enumerate(CPASS):
                    for kp in range(NPAIR):
                        nc.tensor.matmul(
                            pw[0:tsz, cs:cs + cw],
                            lhsT=ou[kp][:, ts:ts + tsz],
                            rhs=w_sb["wp"][kp][:, cs:cs + cw],
                            start=(kp == 0), stop=(kp == NPAIR - 1),
                        )
                obw = sb.tile([128, 768], F32, name="obw", tag="obw", bufs=3)
                nc.vector.tensor_add(obw[0:tsz, 0:768], pw[0:tsz, 0:768],
                                     bias_sb[0:tsz, 0:768])
                nc.sync.dma_start(out_d[it, ts:ts + tsz, 0:768], obw[0:tsz, 0:768])

            # ================= pipeline =================
            state = {}
            for it in range(BPC):
                state[it] = dict(qT={}, kTh={}, v={}, vc={},
                                 ou=[], den=None, amix=None, vmix=None)

            def alloc_item(it):
                s = state[it]
                s["ou"] = [
                    sb.tile([128, 908], BF, name=f"ou_{it}_{p}", tag="ou", bufs=13)
                    for p in range(NPAIR)
                ]
                s["den"] = [
                    sb.tile([6, 908], BF, name=f"den_{it}_{h}", tag="den", bufs=4)
                    for h in range(2)
                ]

            # A0: load + values for item 0 (xT and q/k/p weights DMA after
            # the wv/wvc weights so value projection can start early)
            load_xT(0)
            load_weights(("wvc", "wq", "wk", "wp"))
            nc.sync.dma_start(bias_sb[:], bias_d[:])
            s0 = state[0]
            for c in range(NCH):
                vproj_group(0, c, "wv", "v", s0["v"])
                vproj_group(0, c, "wvc", "vc", s0["vc"])
            s0["amix"], s0["vmix"] = make_mixes(0, s0["v"], s0["vc"])
            alloc_item(0)

            # B phases: attention with q/k lookahead; B0 also streams xT(1),
            # B1 interleaves item0's output projection
            for it in range(BPC):
                s = state[it]
                qkproj(it, 0, s["qT"], s["kTh"])
                op_sched = [1, 1, 1, 1, 1, 1, 1, 1, 1, 1, 0, 0]  # 10 of 16; the
                # remaining 6 run in the tail, overlapping normalize(1B)
                op_done = 0
                for idx in range(12):
                    p, par = idx // 2, idx % 2
                    if par == 0 and p < NPAIR - 1:
                        qkproj(it, p + 1, s["qT"], s["kTh"])
                    attn_iter(it, p, par, s["qT"], s["kTh"], s["v"], s["vc"],
                              s["amix"], s["vmix"], s["ou"], s["den"])
                    if idx == 5:
                        # first-half normalize overlaps the remaining pairs
                        normalize_half(it, s["ou"], s["den"], 0, 3)
                    if it == 0 and idx == 6:
                        load_xT(1)
                    if it == 1:
                        for _ in range(op_sched[idx]):
                            c, cs_i = op_done // 2, op_done % 2
                            outproj_group(0, c, cs_i, state[0]["ou"])
                            op_done += 1
                normalize_half(it, s["ou"], s["den"], 3, 6)
                if it == 0:
                    # A1: values for item 1 (overlaps normalize(0) on PE)
                    s1 = state[1]
                    for c in range(NCH):
                        vproj_group(1, c, "wv", "v", s1["v"])
                        vproj_group(1, c, "wvc", "vc", s1["vc"])
                    s1["amix"], s1["vmix"] = make_mixes(1, s1["v"], s1["vc"])
                    alloc_item(1)

            # C: leftover item-0 groups keep the PE fed while the second
            # normalize half of item 1 drains, then item 1's wide groups
            for g in range(10, 16):
                outproj_group(0, g // 2, g % 2, state[0]["ou"])
            for c in range(NCH):
                outproj_wide(1, c, state[1]["ou"])

    nc.compile()
    return nc


def _get_built():
    global _BUILT
    if _BUILT is None:
        _BUILT = _build()
    return _BUILT


def kernel(x, Wq, Wk, Wv, Wvc, Wp, bp):
    global LAST_RESULTS
    from concourse.bass_utils import run_bass_kernel_spmd

    x = np.asarray(x, dtype=np.float32)
    bf = ml_dtypes.bfloat16
    xT = np.ascontiguousarray(x.transpose(0, 2, 1)).astype(bf)      # (B, C, N)
    ws = {
        "wq": np.asarray(Wq, dtype=np.float32).astype(bf),
        "wk": np.asarray(Wk, dtype=np.float32).astype(bf),
        "wv": np.asarray(Wv, dtype=np.float32).astype(bf),
        "wvc": np.asarray(Wvc, dtype=np.float32).astype(bf),
        "wp": np.asarray(Wp, dtype=np.float32).astype(bf),
    }
    bias = np.ascontiguousarray(
        np.broadcast_to(np.asarray(bp, dtype=np.float32), (128, C))
    )

    if TRACE:
        _install_trace_shim()

    nc = _get_built()
    in_maps = []
    for i in range(N_CORES):
        m = {"xT": np.ascontiguousarray(xT[i * BPC:(i + 1) * BPC]), "bias": bias}
        m.update(ws)
        in_maps.append(m)

    res = run_bass_kernel_spmd(nc, in_maps, list(range(N_CORES)), trace=TRACE,
                               stitch_traces=False)
    LAST_RESULTS = res
    out = np.concatenate([res.results[i]["out"] for i in range(N_CORES)], axis=0)
    return out


# revision 15
# speedup vs baseline: 2.3456x; 1.0217x over previous
"""Multi-modality double-value attention on 8 TRN2 NeuronCores.

Sharding: data-parallel over batch (16 items -> 2 per core). Each core runs
the full attention block for its 2 items; weights are replicated. No
collectives. Host pre-transposes x to x^T and casts inputs to bf16; compute
is bf16 with fp32 PSUM accumulation; output is fp32.

v2 pipeline: the two items are software-pipelined so the PE never idles long
enough for the HAM clock gate to re-throttle. Scores for one (pair, parity)
go into a single 2-bank PSUM tile and are exponentiated with one 906-col
ACT instruction; attention outputs are evacuated to SBUF immediately
(unnormalized, bf16) so PSUM banks recycle fast; softmax division happens
late via one batched reciprocal per item + gpsimd row-broadcasts.
"""

import numpy as np
import ml_dtypes

B, N, C = 16, 906, 768
H = 12
D = 64
M1 = 513
N_CORES = 8
BPC = B // N_CORES          # batch items per core
KC = C // 128               # 6 contraction chunks over C
NPAIR = H // 2              # 6 head pairs
NCH = (N + 127) // 128      # 8 key/token chunks over N
KCH = [(i * 128, min(128, N - i * 128)) for i in range(NCH)]
QP = [(0, 512), (512, N - 512)]      # column passes over N
CPASS = [(0, 512), (512, C - 512)]   # column passes over C
SCALE = D ** -0.5
PW = 194  # per-head-pair value block: [V_e(64) | 1 | 1 | 1 | 0*63 | V_o(64)]

TRACE = False          # set by test.py to capture a HW profile
LAST_RESULTS = None    # BassKernelResults of the most recent run

_BUILT = None


def _install_trace_shim():
    """The image's antenv lacks axon_hooks; recreate it so trace=True works."""
    import sys, types
    if "antenv.axon_hooks" in sys.modules:
        return
    mod = types.ModuleType("antenv.axon_hooks")
    mod._hook = None
    mod.set_axon_ntff_profile_hook = lambda h: setattr(mod, "_hook", h)
    mod.get_axon_ntff_profile_hook = lambda: mod._hook
    sys.modules["antenv.axon_hooks"] = mod
    import antenv
    antenv.axon_hooks = mod
    from trn_agent_boot.trn_boot import _ntff_profile_via_ctypes
    mod.set_axon_ntff_profile_hook(_ntff_profile_via_ctypes("/opt/axon/libaxon_pjrt.so"))


def _build():
    import concourse.tile as tile
    from concourse import bacc, mybir

    BF = mybir.dt.bfloat16
    F32 = mybir.dt.float32
    AF = mybir.ActivationFunctionType

    nc = bacc.Bacc("TRN2", target_bir_lowering=False, debug=False, num_devices=N_CORES)

    xT_d = nc.dram_tensor("xT", [BPC, C, N], BF, kind="ExternalInput").ap()
    w_d = {
        wn: nc.dram_tensor(wn, [C, C], BF, kind="ExternalInput").ap()
        for wn in ("wq", "wk", "wv", "wvc", "wp")
    }
    bias_d = nc.dram_tensor("bias", [128, C], F32, kind="ExternalInput").ap()
    out_d = nc.dram_tensor("out", [BPC, N, C], F32, kind="ExternalOutput").ap()

    with tile.TileContext(nc) as tc:
        from contextlib import ExitStack
        from concourse import library_config

        with ExitStack() as ctx:
            wpool = ctx.enter_context(tc.tile_pool(name="wpool", bufs=1))
            sb = ctx.enter_context(tc.tile_pool(name="sb", bufs=1))
            ps = ctx.enter_context(tc.tile_pool(name="ps", bufs=1, space="PSUM"))

            # partition_broadcast lives in the gpsimd 'attn' library; the
            # default 'standard' library executes it as garbage on HW
            nc.gpsimd.load_library(library_config.attn)

            # ---- constants: weights + bias ----
            # DMA order matters for the pipeline head: the first compute
            # phase (vproj of item 0) needs wv/wvc, so load those first
            w_sb = {}

            def load_weights(names):
                for wn in names:
                    tiles = []
                    for kc in range(KC):
                        t = wpool.tile([128, C], BF, name=f"{wn}_{kc}", tag=f"{wn}_{kc}")
                        nc.sync.dma_start(t[:], w_d[wn][kc * 128:(kc + 1) * 128, :])
                        tiles.append(t)
                    w_sb[wn] = tiles

            load_weights(("wv",))
            bias_sb = wpool.tile([128, C], F32, name="bias_sb", tag="bias_sb")

            # ---- rotating state shared across the two items ----
            xT = {}     # (it, kc) -> tile

            def load_xT(it):
                for kc in range(KC):
                    t = sb.tile([128, N], BF, name=f"xT_{it}_{kc}", tag="xT", bufs=8)
                    nc.sync.dma_start(t[:], xT_d[it, kc * 128:(kc + 1) * 128, :])
                    xT[(it, kc)] = t

            # ---------- projection helpers ----------
            def qkproj(it, t_, qT, kTh):
                """q and k projections for head pair t_ of item it."""
                dst = sb.tile([128, N], BF, name=f"qT_{it}_{t_}", tag="qT", bufs=4)
                for (qs, qw) in QP:
                    pp = ps.tile([128, 512], F32, name="pp", tag="pp", bufs=2)
                    for kc in range(KC):
                        nc.tensor.matmul(
                            pp[:, 0:qw],
                            lhsT=w_sb["wq"][kc][:, t_ * 128:(t_ + 1) * 128],
                            rhs=xT[(it, kc)][:, qs:qs + qw],
                            start=(kc == 0), stop=(kc == KC - 1),
                        )
                    # explicit DVE: ACT is saturated by exp during B phases
                    nc.vector.tensor_copy(dst[:, qs:qs + qw], pp[:, 0:qw])
                qT[t_] = dst
                # k^T per head, zero-padded to 128 partitions so S^T runs as a
                # plain K=128 matmul (no PE row tiling -- T8 tile corrupts on HW)
                ke = sb.tile([128, N], BF, name=f"kTh_{it}_{2*t_}", tag="kT", bufs=6)
                ko = sb.tile([128, N], BF, name=f"kTh_{it}_{2*t_+1}", tag="kT", bufs=6)
                nc.vector.memset(ke[64:128, :], 0.0)
                nc.vector.memset(ko[0:64, :], 0.0)
                for (qs, qw) in QP:
                    pp = ps.tile([128, 512], F32, name="pp", tag="pp", bufs=2)
                    for kc in range(KC):
                        nc.tensor.matmul(
                            pp[:, 0:qw],
                            lhsT=w_sb["wk"][kc][:, t_ * 128:(t_ + 1) * 128],
                            rhs=xT[(it, kc)][:, qs:qs + qw],
                            start=(kc == 0), stop=(kc == KC - 1),
                        )
                    nc.vector.tensor_copy(ke[0:64, qs:qs + qw], pp[0:64, 0:qw])
                    nc.vector.tensor_copy(ko[64:128, qs:qs + qw], pp[64:128, 0:qw])
                kTh[2 * t_] = ke
                kTh[2 * t_ + 1] = ko

            def vproj_group(it, c, wn, tg, dst_map):
                """values for key chunk c of item it, matrix wn (wv/wvc)."""
                ts, tsz = KCH[c]
                dst = sb.tile([128, NPAIR * PW], BF, name=f"{tg}_{it}_{c}",
                              tag=tg, bufs=9)
                if tsz < 128:
                    # stationary loads may read all 128 partitions; keep
                    # the unwritten tail finite
                    nc.vector.memset(dst[:, :], 0.0)
                dvw = dst[0:tsz, :].rearrange("p (g c) -> p g c", c=PW)
                for (cs, cw) in CPASS:
                    pp = ps.tile([128, 512], F32, name="pp", tag="pp", bufs=2)
                    for kc in range(KC):
                        nc.tensor.matmul(
                            pp[0:tsz, 0:cw],
                            lhsT=xT[(it, kc)][:, ts:ts + tsz],
                            rhs=w_sb[wn][kc][:, cs:cs + cw],
                            start=(kc == 0), stop=(kc == KC - 1),
                        )
                    g0, gn = (0, 4) if cs == 0 else (4, 2)
                    src = pp[0:tsz, 0:cw].rearrange("p (g r d) -> p g r d", r=2, d=D)
                    nc.any.tensor_copy(dvw[:, g0:g0 + gn, 0:D], src[:, :, 0, :])
                    nc.any.tensor_copy(dvw[:, g0:g0 + gn, 130:194], src[:, :, 1, :])
                nc.vector.memset(dvw[:, :, 64:67], 1.0)
                nc.vector.memset(dvw[:, :, 67:130], 0.0)
                dst_map[c] = dst

            def make_mixes(it, v_sb, vc_sb):
                # mixed tiles for the key chunk straddling M1 (chunk 4: key 512
                # is modality-a, keys 513.. are modality-v)
                amix = sb.tile([128, NPAIR * PW], BF, name=f"amix_{it}", tag="amix", bufs=2)
                vmix = sb.tile([128, NPAIR * PW], BF, name=f"vmix_{it}", tag="vmix", bufs=2)
                nc.vector.tensor_copy(amix[:, :], vc_sb[4][:, :])
                nc.vector.tensor_copy(amix[0:1, :], v_sb[4][0:1, :])
                nc.vector.tensor_copy(vmix[:, :], v_sb[4][:, :])
                nc.vector.tensor_copy(vmix[0:1, :], vc_sb[4][0:1, :])
                return amix, vmix

            # ---------- attention iteration ----------
            def attn_iter(it, p, par, qT, kTh, v_sb, vc_sb, amix, vmix, ou, den):
                # S^T (keys on partitions) one key chunk at a time, into a
                # 2-bank PSUM tile; one exp per chunk covering all 906 queries
                exps = []
                for c, (ks, ksz) in enumerate(KCH):
                    sc = ps.tile([128, 1024], F32, name="sc", tag="sc", bufs=2)
                    nc.tensor.matmul(sc[0:ksz, 0:512],
                                     lhsT=kTh[2 * p + par][:, ks:ks + ksz],
                                     rhs=qT[p][:, 0:512], start=True, stop=True)
                    nc.tensor.matmul(sc[0:ksz, 512:906],
                                     lhsT=kTh[2 * p + par][:, ks:ks + ksz],
                                     rhs=qT[p][:, 512:906], start=True, stop=True)
                    e = sb.tile([128, 908], BF, name="ee", tag="ee", bufs=9)
                    nc.scalar.activation(e[0:ksz, 0:906], sc[0:ksz, 0:906],
                                         AF.Exp, scale=SCALE)
                    exps.append(e)

                if par == 0:
                    rows = slice(0, 65)
                    csl = slice(p * PW, p * PW + 65)          # [V_even | 1]
                    drow, orows = 64, slice(0, 64)
                else:
                    rows = slice(0, 128)
                    csl = slice(p * PW + 66, p * PW + PW)     # [1 | 0*63 | V_odd]
                    drow, orows = 0, slice(64, 128)

                t1 = ps.tile([128, 512], F32, name="t1", tag="t1", bufs=1)
                t2 = ps.tile([128, 512], F32, name="t2", tag="t2", bufs=1)

                def va(c):
                    return amix if c == 4 else (v_sb[c] if c < 4 else vc_sb[c])

                def vv(c):
                    return vmix if c == 4 else (vc_sb[c] if c < 4 else v_sb[c])

                # modality-a queries q in [0,512)
                for c, (ks, ksz) in enumerate(KCH):
                    nc.tensor.matmul(t1[rows, 0:512], lhsT=va(c)[0:ksz, csl],
                                     rhs=exps[c][0:ksz, 0:512],
                                     start=(c == 0), stop=(c == NCH - 1))
                # modality-v queries q in [512,906) (col 0 of the block, q=512,
                # is modality-a and gets fixed up by the racc column), plus the
                # q=512 a-modality column accumulated into col 400 of the same
                # bank as one extended accumulation group (ordered by the
                # critical section; racc matmuls carry start=False so they
                # accumulate under o2's group clear)
                with tc.tile_critical():
                    for c, (ks, ksz) in enumerate(KCH):
                        nc.tensor.matmul(t2[rows, 0:394], lhsT=vv(c)[0:ksz, csl],
                                         rhs=exps[c][0:ksz, 512:906],
                                         start=(c == 0), stop=False)
                    for c, (ks, ksz) in enumerate(KCH):
                        nc.tensor.matmul(t2[rows, 400:401], lhsT=va(c)[0:ksz, csl],
                                         rhs=exps[c][0:ksz, 512:513],
                                         start=False, stop=(c == NCH - 1))

                # evacuate PSUM immediately (unnormalized, bf16); denominators
                # go to a staging row then DMA into the per-item gather tile
                j = 2 * p + par
                ob = ou[p]
                nc.vector.tensor_copy(ob[orows, 0:512], t1[orows, 0:512])
                nc.vector.tensor_copy(ob[orows, 512:906], t2[orows, 0:394])
                nc.vector.tensor_copy(ob[orows, 512:513], t2[orows, 400:401])
                dstage = sb.tile([128, 908], BF, name="dstage", tag="dstage", bufs=2)
                dr = slice(drow, drow + 1)
                nc.vector.tensor_copy(dstage[dr, 0:512], t1[dr, 0:512])
                nc.vector.tensor_copy(dstage[dr, 513:906], t2[dr, 1:394])
                nc.vector.tensor_copy(dstage[dr, 512:513], t2[dr, 400:401])
                dh = den[j // 6]
                r6 = j % 6
                nc.sync.dma_start(dh[r6:r6 + 1, 0:906], dstage[dr, 0:906])

            def normalize_half(it, ou, den, plo, phi):
                # batched reciprocal for pairs [plo, phi), then per-row
                # broadcast + in-place multiply; called per half-item so the
                # chain overlaps the remaining attention iterations
                dh = den[(2 * plo) // 6]
                with nc.allow_low_precision(reason="softmax recip in bf16"):
                    for (ca, cb) in ((0, 302), (302, 604), (604, 906)):
                        nc.vector.reciprocal(dh[0:6, ca:cb], dh[0:6, ca:cb])
                for p in range(plo, phi):
                    for par in range(2):
                        j = 2 * p + par
                        r6 = j % 6
                        orows = slice(0, 64) if par == 0 else slice(64, 128)
                        stg = sb.tile([1, 908], BF, name="stg", tag="stg", bufs=4)
                        nc.sync.dma_start(stg[0:1, 0:906], dh[r6:r6 + 1, 0:906])
                        bc2 = sb.tile([128, 908], BF, name="bc2", tag="bc2", bufs=4)
                        nc.gpsimd.partition_broadcast(bc2[:, 0:906], stg[0:1, 0:906])
                        nc.any.tensor_mul(ou[p][orows, 0:906], ou[p][orows, 0:906],
                                          bc2[orows, 0:906])

            def outproj_group(it, c, cs_i, ou, ptag="pp", pbufs=2):
                ts, tsz = KCH[c]
                cs, cw = CPASS[cs_i]
                pp = ps.tile([128, 512], F32, name="pp", tag=ptag, bufs=pbufs)
                for kp in range(NPAIR):
                    nc.tensor.matmul(
                        pp[0:tsz, 0:cw],
                        lhsT=ou[kp][:, ts:ts + tsz],
                        rhs=w_sb["wp"][kp][:, cs:cs + cw],
                        start=(kp == 0), stop=(kp == NPAIR - 1),
                    )
                obt = sb.tile([128, 512], F32, name="obt", tag="obt", bufs=2)
                nc.vector.tensor_add(obt[0:tsz, 0:cw], pp[0:tsz, 0:cw],
                                     bias_sb[0:tsz, cs:cs + cw])
                nc.sync.dma_start(out_d[it, ts:ts + tsz, cs:cs + cw], obt[0:tsz, 0:cw])

            def outproj_wide(it, c, ou):
                # tail variant: both column passes into one 2-bank PSUM tile
                # (the sc tag is free once attention is done), single bias add
                # and single full-row DMA -- fewer serialization points
                ts, tsz = KCH[c]
                pw = ps.tile([128, 1024], F32, name="sc", tag="sc", bufs=2)
                for cs_i, (cs, cw) in enumerate(CPASS):
                    for kp in range(NPAIR):
                        nc.tensor.matmul(
                            pw[0:tsz, cs:cs + cw],
                            lhsT=ou[kp][:, ts:ts + tsz],
                            rhs=w_sb["wp"][kp][:, cs:cs + cw],
                            start=(kp == 0), stop=(kp == NPAIR - 1),
                        )
                obw = sb.tile([128, 768], F32, name="obw", tag="obw", bufs=3)
                nc.vector.tensor_add(obw[0:tsz, 0:768], pw[0:tsz, 0:768],
                                     bias_sb[0:tsz, 0:768])
                nc.sync.dma_start(out_d[it, ts:ts + tsz, 0:768], obw[0:tsz, 0:768])

            # ================= pipeline =================
            state = {}
            for it in range(BPC):
                state[it] = dict(qT={}, kTh={}, v={}, vc={},
                                 ou=[], den=None, amix=None, vmix=None)

            def alloc_item(it):
                s = state[it]
                s["ou"] = [
                    sb.tile([128, 908], BF, name=f"ou_{it}_{p}", tag="ou", bufs=13)
                    for p in range(NPAIR)
                ]
                s["den"] = [
                    sb.tile([6, 908], BF, name=f"den_{it}_{h}", tag="den", bufs=4)
                    for h in range(2)
                ]

            # A0: load + values for item 0 (xT and q/k/p weights DMA after
            # the wv/wvc weights so value projection can start early)
            load_xT(0)
            load_weights(("wvc", "wq", "wk", "wp"))
            nc.sync.dma_start(bias_sb[:], bias_d[:])
            s0 = state[0]
            for c in range(NCH):
                vproj_group(0, c, "wv", "v", s0["v"])
                vproj_group(0, c, "wvc", "vc", s0["vc"])
            s0["amix"], s0["vmix"] = make_mixes(0, s0["v"], s0["vc"])
            alloc_item(0)

            # B phases: attention with q/k lookahead; B0 also streams xT(1),
            # B1 interleaves item0's output projection
            for it in range(BPC):
                s = state[it]
                qkproj(it, 0, s["qT"], s["kTh"])
                op_sched = [1, 1, 1, 1, 1, 1, 1, 1, 1, 1, 0, 0]  # 10 of 16; the
                # remaining 6 run in the tail, overlapping normalize(1B)
                op_done = 0
                for idx in range(12):
                    p, par = idx // 2, idx % 2
                    if par == 0 and p < NPAIR - 1:
                        qkproj(it, p + 1, s["qT"], s["kTh"])
                    attn_iter(it, p, par, s["qT"], s["kTh"], s["v"], s["vc"],
                              s["amix"], s["vmix"], s["ou"], s["den"])
                    if idx == 5 and it == 1:
                        # first-half normalize overlaps the remaining pairs
                        normalize_half(it, s["ou"], s["den"], 0, 3)
                    if it == 0 and idx == 6:
                        load_xT(1)
                    if it == 1:
                        for _ in range(op_sched[idx]):
                            c, cs_i = op_done // 2, op_done % 2
                            outproj_group(0, c, cs_i, state[0]["ou"])
                            op_done += 1
                if it == 0:
                    normalize_half(it, s["ou"], s["den"], 0, 3)
                normalize_half(it, s["ou"], s["den"], 3, 6)
                if it == 0:
                    # A1: values for item 1 (overlaps normalize(0) on PE)
                    s1 = state[1]
                    for c in range(NCH):
                        vproj_group(1, c, "wv", "v", s1["v"])
                        vproj_group(1, c, "wvc", "vc", s1["vc"])
                    s1["amix"], s1["vmix"] = make_mixes(1, s1["v"], s1["vc"])
                    alloc_item(1)

            # C: leftover item-0 groups keep the PE fed while the second
            # normalize half of item 1 drains, then item 1's wide groups
            for g in range(10, 16):
                outproj_group(0, g // 2, g % 2, state[0]["ou"],
                              ptag=("t1" if g % 2 == 0 else "t2"), pbufs=1)
            for c in range(NCH):
                outproj_wide(1, c, state[1]["ou"])

    nc.compile()
    return nc


def _get_built():
    global _BUILT
    if _BUILT is None:
        _BUILT = _build()
    return _BUILT


def kernel(x, Wq, Wk, Wv, Wvc, Wp, bp):
    global LAST_RESULTS
    from concourse.bass_utils import run_bass_kernel_spmd

    x = np.asarray(x, dtype=np.float32)
    bf = ml_dtypes.bfloat16
    xT = np.ascontiguousarray(x.transpose(0, 2, 1)).astype(bf)      # (B, C, N)
    ws = {
        "wq": np.asarray(Wq, dtype=np.float32).astype(bf),
        "wk": np.asarray(Wk, dtype=np.float32).astype(bf),
        "wv": np.asarray(Wv, dtype=np.float32).astype(bf),
        "wvc": np.asarray(Wvc, dtype=np.float32).astype(bf),
        "wp": np.asarray(Wp, dtype=np.float32).astype(bf),
    }
    bias = np.ascontiguousarray(
        np.broadcast_to(np.asarray(bp, dtype=np.float32), (128, C))
    )

    if TRACE:
        _install_trace_shim()

    nc = _get_built()
    in_maps = []
    for i in range(N_CORES):
        m = {"xT": np.ascontiguousarray(xT[i * BPC:(i + 1) * BPC]), "bias": bias}
        m.update(ws)
        in_maps.append(m)

    res = run_bass_kernel_spmd(nc, in_maps, list(range(N_CORES)), trace=TRACE,
                               stitch_traces=False)
    LAST_RESULTS = res
    out = np.concatenate([res.results[i]["out"] for i in range(N_CORES)], axis=0)
    return out


# revision 17
# speedup vs baseline: 2.4281x; 1.0351x over previous
"""Multi-modality double-value attention on 8 TRN2 NeuronCores.

Sharding: data-parallel over batch (16 items -> 2 per core). Each core runs
the full attention block for its 2 items; weights are replicated. No
collectives. Host pre-transposes x to x^T and casts inputs to bf16; compute
is bf16 with fp32 PSUM accumulation; output is fp32.

v2 pipeline: the two items are software-pipelined so the PE never idles long
enough for the HAM clock gate to re-throttle. Scores for one (pair, parity)
go into a single 2-bank PSUM tile and are exponentiated with one 906-col
ACT instruction; attention outputs are evacuated to SBUF immediately
(unnormalized, bf16) so PSUM banks recycle fast; softmax division happens
late via one batched reciprocal per item + gpsimd row-broadcasts.
"""

import numpy as np
import ml_dtypes

B, N, C = 16, 906, 768
H = 12
D = 64
M1 = 513
N_CORES = 8
BPC = B // N_CORES          # batch items per core
KC = C // 128               # 6 contraction chunks over C
NPAIR = H // 2              # 6 head pairs
NCH = (N + 127) // 128      # 8 key/token chunks over N
KCH = [(i * 128, min(128, N - i * 128)) for i in range(NCH)]
QP = [(0, 512), (512, N - 512)]      # column passes over N
CPASS = [(0, 512), (512, C - 512)]   # column passes over C
SCALE = D ** -0.5
PW = 194  # per-head-pair value block: [V_e(64) | 1 | 1 | 1 | 0*63 | V_o(64)]

TRACE = False          # set by test.py to capture a HW profile
LAST_RESULTS = None    # BassKernelResults of the most recent run

_BUILT = None


def _install_trace_shim():
    """The image's antenv lacks axon_hooks; recreate it so trace=True works."""
    import sys, types
    if "antenv.axon_hooks" in sys.modules:
        return
    mod = types.ModuleType("antenv.axon_hooks")
    mod._hook = None
    mod.set_axon_ntff_profile_hook = lambda h: setattr(mod, "_hook", h)
    mod.get_axon_ntff_profile_hook = lambda: mod._hook
    sys.modules["antenv.axon_hooks"] = mod
    import antenv
    antenv.axon_hooks = mod
    from trn_agent_boot.trn_boot import _ntff_profile_via_ctypes
    mod.set_axon_ntff_profile_hook(_ntff_profile_via_ctypes("/opt/axon/libaxon_pjrt.so"))


def _build():
    import concourse.tile as tile
    from concourse import bacc, mybir

    BF = mybir.dt.bfloat16
    F32 = mybir.dt.float32
    AF = mybir.ActivationFunctionType

    nc = bacc.Bacc("TRN2", target_bir_lowering=False, debug=False, num_devices=N_CORES)

    xT_d = nc.dram_tensor("xT", [BPC, C, N], BF, kind="ExternalInput").ap()
    w_d = {
        wn: nc.dram_tensor(wn, [C, C], BF, kind="ExternalInput").ap()
        for wn in ("wq", "wk", "wv", "wvc", "wp")
    }
    bias_d = nc.dram_tensor("bias", [128, C], F32, kind="ExternalInput").ap()
    out_d = nc.dram_tensor("out", [BPC, N, C], F32, kind="ExternalOutput").ap()

    with tile.TileContext(nc) as tc:
        from contextlib import ExitStack
        from concourse import library_config

        with ExitStack() as ctx:
            wpool = ctx.enter_context(tc.tile_pool(name="wpool", bufs=1))
            sb = ctx.enter_context(tc.tile_pool(name="sb", bufs=1))
            ps = ctx.enter_context(tc.tile_pool(name="ps", bufs=1, space="PSUM"))

            # partition_broadcast lives in the gpsimd 'attn' library; the
            # default 'standard' library executes it as garbage on HW
            nc.gpsimd.load_library(library_config.attn)

            # ---- constants: weights + bias ----
            # DMA order matters for the pipeline head: the first compute
            # phase (vproj of item 0) needs wv/wvc, so load those first
            w_sb = {}

            def load_weights(names):
                for wn in names:
                    tiles = []
                    for kc in range(KC):
                        t = wpool.tile([128, C], BF, name=f"{wn}_{kc}", tag=f"{wn}_{kc}")
                        nc.sync.dma_start(t[:], w_d[wn][kc * 128:(kc + 1) * 128, :])
                        tiles.append(t)
                    w_sb[wn] = tiles

            load_weights(("wv",))
            bias_sb = wpool.tile([128, C], F32, name="bias_sb", tag="bias_sb")

            # ---- rotating state shared across the two items ----
            xT = {}     # (it, kc) -> tile

            def load_xT(it):
                for kc in range(KC):
                    t = sb.tile([128, N], BF, name=f"xT_{it}_{kc}", tag="xT", bufs=8)
                    nc.sync.dma_start(t[:], xT_d[it, kc * 128:(kc + 1) * 128, :])
                    xT[(it, kc)] = t

            # ---------- projection helpers ----------
            def qkproj(it, t_, qT, kTh):
                """q and k projections for head pair t_ of item it."""
                dst = sb.tile([128, N], BF, name=f"qT_{it}_{t_}", tag="qT", bufs=3)
                for (qs, qw) in QP:
                    pp = ps.tile([128, 512], F32, name="pp", tag="pp", bufs=2)
                    for kc in range(KC):
                        nc.tensor.matmul(
                            pp[:, 0:qw],
                            lhsT=w_sb["wq"][kc][:, t_ * 128:(t_ + 1) * 128],
                            rhs=xT[(it, kc)][:, qs:qs + qw],
                            start=(kc == 0), stop=(kc == KC - 1),
                        )
                    # explicit DVE: ACT is saturated by exp during B phases
                    nc.vector.tensor_copy(dst[:, qs:qs + qw], pp[:, 0:qw])
                qT[t_] = dst
                # k^T per head, zero-padded to 128 partitions so S^T runs as a
                # plain K=128 matmul (no PE row tiling -- T8 tile corrupts on HW)
                ke = sb.tile([128, N], BF, name=f"kTh_{it}_{2*t_}", tag="kT", bufs=6)
                ko = sb.tile([128, N], BF, name=f"kTh_{it}_{2*t_+1}", tag="kT", bufs=6)
                nc.vector.memset(ke[64:128, :], 0.0)
                nc.vector.memset(ko[0:64, :], 0.0)
                for (qs, qw) in QP:
                    pp = ps.tile([128, 512], F32, name="pp", tag="pp", bufs=2)
                    for kc in range(KC):
                        nc.tensor.matmul(
                            pp[:, 0:qw],
                            lhsT=w_sb["wk"][kc][:, t_ * 128:(t_ + 1) * 128],
                            rhs=xT[(it, kc)][:, qs:qs + qw],
                            start=(kc == 0), stop=(kc == KC - 1),
                        )
                    nc.vector.tensor_copy(ke[0:64, qs:qs + qw], pp[0:64, 0:qw])
                    nc.vector.tensor_copy(ko[64:128, qs:qs + qw], pp[64:128, 0:qw])
                kTh[2 * t_] = ke
                kTh[2 * t_ + 1] = ko

            def vproj_group(it, c, wn, tg, dst_map):
                """values for key chunk c of item it, matrix wn (wv/wvc)."""
                ts, tsz = KCH[c]
                dst = sb.tile([128, NPAIR * PW], BF, name=f"{tg}_{it}_{c}",
                              tag=tg, bufs=9)
                if tsz < 128:
                    # stationary loads may read all 128 partitions; keep
                    # the unwritten tail finite
                    nc.vector.memset(dst[:, :], 0.0)
                dvw = dst[0:tsz, :].rearrange("p (g c) -> p g c", c=PW)
                for (cs, cw) in CPASS:
                    pp = ps.tile([128, 512], F32, name="pp", tag="pp", bufs=2)
                    for kc in range(KC):
                        nc.tensor.matmul(
                            pp[0:tsz, 0:cw],
                            lhsT=xT[(it, kc)][:, ts:ts + tsz],
                            rhs=w_sb[wn][kc][:, cs:cs + cw],
                            start=(kc == 0), stop=(kc == KC - 1),
                        )
                    g0, gn = (0, 4) if cs == 0 else (4, 2)
                    src = pp[0:tsz, 0:cw].rearrange("p (g r d) -> p g r d", r=2, d=D)
                    nc.any.tensor_copy(dvw[:, g0:g0 + gn, 0:D], src[:, :, 0, :])
                    nc.any.tensor_copy(dvw[:, g0:g0 + gn, 130:194], src[:, :, 1, :])
                nc.vector.memset(dvw[:, :, 64:67], 1.0)
                nc.vector.memset(dvw[:, :, 67:130], 0.0)
                dst_map[c] = dst

            def make_mixes(it, v_sb, vc_sb):
                # mixed tiles for the key chunk straddling M1 (chunk 4: key 512
                # is modality-a, keys 513.. are modality-v)
                amix = sb.tile([128, NPAIR * PW], BF, name=f"amix_{it}", tag="amix", bufs=2)
                vmix = sb.tile([128, NPAIR * PW], BF, name=f"vmix_{it}", tag="vmix", bufs=2)
                nc.vector.tensor_copy(amix[:, :], vc_sb[4][:, :])
                nc.vector.tensor_copy(amix[0:1, :], v_sb[4][0:1, :])
                nc.vector.tensor_copy(vmix[:, :], v_sb[4][:, :])
                nc.vector.tensor_copy(vmix[0:1, :], vc_sb[4][0:1, :])
                return amix, vmix

            # ---------- attention iteration ----------
            # Emission is software-pipelined: the o2 accumulation's middle
            # matmuls and the t2-side evacuations of iteration i are emitted
            # in the middle of iteration i+1's score stream (via the returned
            # closure), so the next iteration's first score matmul follows the
            # critical section immediately and the ACT exp stream never stalls.
            def attn_iter(it, p, par, qT, kTh, v_sb, vc_sb, amix, vmix, ou, den,
                          prev_close, mid_emit=None):
                exps = []

                def do_chunk(c):
                    ks, ksz = KCH[c]
                    sc = ps.tile([128, 1024], F32, name="sc", tag="sc", bufs=2)
                    nc.tensor.matmul(sc[0:ksz, 0:512],
                                     lhsT=kTh[2 * p + par][:, ks:ks + ksz],
                                     rhs=qT[p][:, 0:512], start=True, stop=True)
                    nc.tensor.matmul(sc[0:ksz, 512:906],
                                     lhsT=kTh[2 * p + par][:, ks:ks + ksz],
                                     rhs=qT[p][:, 512:906], start=True, stop=True)
                    e = sb.tile([128, 908], BF, name="ee", tag="ee", bufs=12)
                    nc.scalar.activation(e[0:ksz, 0:906], sc[0:ksz, 0:906],
                                         AF.Exp, scale=SCALE)
                    exps.append(e)

                for c in range(4):
                    do_chunk(c)
                if prev_close is not None:
                    prev_close()
                if mid_emit is not None:
                    mid_emit()
                for c in range(4, NCH):
                    do_chunk(c)

                if par == 0:
                    rows = slice(0, 65)
                    csl = slice(p * PW, p * PW + 65)          # [V_even | 1]
                    drow, orows = 64, slice(0, 64)
                else:
                    rows = slice(0, 128)
                    csl = slice(p * PW + 66, p * PW + PW)     # [1 | 0*63 | V_odd]
                    drow, orows = 0, slice(64, 128)

                t1 = ps.tile([128, 512], F32, name="t1", tag="t1", bufs=1)
                t2 = ps.tile([128, 512], F32, name="t2", tag="t2", bufs=1)

                def va(c):
                    return amix if c == 4 else (v_sb[c] if c < 4 else vc_sb[c])

                def vv(c):
                    return vmix if c == 4 else (vc_sb[c] if c < 4 else v_sb[c])

                # modality-a queries q in [0,512)
                for c, (ks, ksz) in enumerate(KCH):
                    nc.tensor.matmul(t1[rows, 0:512], lhsT=va(c)[0:ksz, csl],
                                     rhs=exps[c][0:ksz, 0:512],
                                     start=(c == 0), stop=(c == NCH - 1))

                # t1-side evacuation (unnormalized, bf16) + denominator row
                j = 2 * p + par
                ob = ou[p]
                dstage = sb.tile([128, 908], BF, name="dstage", tag="dstage", bufs=3)
                dr = slice(drow, drow + 1)
                nc.vector.tensor_copy(ob[orows, 0:512], t1[orows, 0:512])
                nc.vector.tensor_copy(dstage[dr, 0:512], t1[dr, 0:512])

                # open the t2 accumulation group: its start matmul plus the
                # q=512 column accumulators (start=False so they extend the
                # group; the critical section pins the start matmul first)
                with tc.tile_critical():
                    nc.tensor.matmul(t2[rows, 0:394], lhsT=vv(0)[0:128, csl],
                                     rhs=exps[0][0:128, 512:906],
                                     start=True, stop=False)
                    for c, (ks, ksz) in enumerate(KCH):
                        nc.tensor.matmul(t2[rows, 400:401], lhsT=va(c)[0:ksz, csl],
                                         rhs=exps[c][0:ksz, 512:513],
                                         start=False, stop=False)

                def close():
                    # o2 middles (emitted during the next iteration's scores)
                    for c in range(1, NCH):
                        ks, ksz = KCH[c]
                        nc.tensor.matmul(t2[rows, 0:394], lhsT=vv(c)[0:ksz, csl],
                                         rhs=exps[c][0:ksz, 512:906],
                                         start=False, stop=(c == NCH - 1))
                    nc.vector.tensor_copy(ob[orows, 512:906], t2[orows, 0:394])
                    nc.vector.tensor_copy(ob[orows, 512:513], t2[orows, 400:401])
                    nc.vector.tensor_copy(dstage[dr, 513:906], t2[dr, 1:394])
                    nc.vector.tensor_copy(dstage[dr, 512:513], t2[dr, 400:401])
                    dh = den[j // 6]
                    r6 = j % 6
                    nc.sync.dma_start(dh[r6:r6 + 1, 0:906], dstage[dr, 0:906])

                return close

            def normalize_half(it, ou, den, plo, phi):
                # batched reciprocal for pairs [plo, phi), then per-row
                # broadcast + in-place multiply; called per half-item so the
                # chain overlaps the remaining attention iterations
                dh = den[(2 * plo) // 6]
                with nc.allow_low_precision(reason="softmax recip in bf16"):
                    for (ca, cb) in ((0, 302), (302, 604), (604, 906)):
                        nc.vector.reciprocal(dh[0:6, ca:cb], dh[0:6, ca:cb])
                for p in range(plo, phi):
                    for par in range(2):
                        j = 2 * p + par
                        r6 = j % 6
                        orows = slice(0, 64) if par == 0 else slice(64, 128)
                        stg = sb.tile([1, 908], BF, name="stg", tag="stg", bufs=3)
                        nc.sync.dma_start(stg[0:1, 0:906], dh[r6:r6 + 1, 0:906])
                        bc2 = sb.tile([128, 908], BF, name="bc2", tag="bc2", bufs=3)
                        nc.gpsimd.partition_broadcast(bc2[:, 0:906], stg[0:1, 0:906])
                        nc.any.tensor_mul(ou[p][orows, 0:906], ou[p][orows, 0:906],
                                          bc2[orows, 0:906])

            def outproj_group(it, c, cs_i, ou, ptag="pp", pbufs=2):
                ts, tsz = KCH[c]
                cs, cw = CPASS[cs_i]
                pp = ps.tile([128, 512], F32, name="pp", tag=ptag, bufs=pbufs)
                for kp in range(NPAIR):
                    nc.tensor.matmul(
                        pp[0:tsz, 0:cw],
                        lhsT=ou[kp][:, ts:ts + tsz],
                        rhs=w_sb["wp"][kp][:, cs:cs + cw],
                        start=(kp == 0), stop=(kp == NPAIR - 1),
                    )
                obt = sb.tile([128, 512], F32, name="obt", tag="obt", bufs=2)
                nc.vector.tensor_add(obt[0:tsz, 0:cw], pp[0:tsz, 0:cw],
                                     bias_sb[0:tsz, cs:cs + cw])
                nc.sync.dma_start(out_d[it, ts:ts + tsz, cs:cs + cw], obt[0:tsz, 0:cw])

            def outproj_wide(it, c, ou):
                # tail variant: both column passes into one 2-bank PSUM tile
                # (the sc tag is free once attention is done), single bias add
                # and single full-row DMA -- fewer serialization points
                ts, tsz = KCH[c]
                pw = ps.tile([128, 1024], F32, name="sc", tag="sc", bufs=2)
                for cs_i, (cs, cw) in enumerate(CPASS):
                    for kp in range(NPAIR):
                        nc.tensor.matmul(
                            pw[0:tsz, cs:cs + cw],
                            lhsT=ou[kp][:, ts:ts + tsz],
                            rhs=w_sb["wp"][kp][:, cs:cs + cw],
                            start=(kp == 0), stop=(kp == NPAIR - 1),
                        )
                obw = sb.tile([128, 768], F32, name="obw", tag="obw", bufs=2)
                nc.vector.tensor_add(obw[0:tsz, 0:768], pw[0:tsz, 0:768],
                                     bias_sb[0:tsz, 0:768])
                nc.sync.dma_start(out_d[it, ts:ts + tsz, 0:768], obw[0:tsz, 0:768])

            # ================= pipeline =================
            state = {}
            for it in range(BPC):
                state[it] = dict(qT={}, kTh={}, v={}, vc={},
                                 ou=[], den=None, amix=None, vmix=None)

            def alloc_item(it):
                s = state[it]
                s["ou"] = [
                    sb.tile([128, 908], BF, name=f"ou_{it}_{p}", tag="ou", bufs=13)
                    for p in range(NPAIR)
                ]
                s["den"] = [
                    sb.tile([6, 908], BF, name=f"den_{it}_{h}", tag="den", bufs=4)
                    for h in range(2)
                ]

            # A0: load + values for item 0 (xT and q/k/p weights DMA after
            # the wv/wvc weights so value projection can start early)
            load_xT(0)
            load_weights(("wvc", "wq", "wk", "wp"))
            nc.sync.dma_start(bias_sb[:], bias_d[:])
            s0 = state[0]
            for c in range(NCH):
                vproj_group(0, c, "wv", "v", s0["v"])
                vproj_group(0, c, "wvc", "vc", s0["vc"])
            s0["amix"], s0["vmix"] = make_mixes(0, s0["v"], s0["vc"])
            alloc_item(0)

            # B phases: attention with q/k lookahead; B0 also streams xT(1),
            # B1 interleaves item0's output projection
            for it in range(BPC):
                s = state[it]
                qkproj(it, 0, s["qT"], s["kTh"])
                op_sched = [1, 1, 1, 1, 1, 1, 1, 1, 1, 1, 0, 0]  # 10 of 16; the
                # remaining 6 run in the tail, overlapping normalize(1B)
                op_state = {"done": 0}
                prev_close = None
                for idx in range(12):
                    p, par = idx // 2, idx % 2

                    def mid_emit(idx=idx, p=p, par=par):
                        if par == 0 and p < NPAIR - 1:
                            qkproj(it, p + 1, s["qT"], s["kTh"])
                        if it == 0 and idx == 6:
                            load_xT(1)
                        if it == 1:
                            for _ in range(op_sched[idx]):
                                g = op_state["done"]
                                outproj_group(0, g // 2, g % 2, state[0]["ou"])
                                op_state["done"] += 1
                        if idx == 6 and it == 1:
                            normalize_half(it, s["ou"], s["den"], 0, 3)

                    prev_close = attn_iter(it, p, par, s["qT"], s["kTh"],
                                           s["v"], s["vc"], s["amix"], s["vmix"],
                                           s["ou"], s["den"], prev_close, mid_emit)
                prev_close()
                if it == 0:
                    normalize_half(it, s["ou"], s["den"], 0, 3)
                normalize_half(it, s["ou"], s["den"], 3, 6)
                if it == 0:
                    # A1: values for item 1 (overlaps normalize(0) on PE)
                    s1 = state[1]
                    for c in range(NCH):
                        vproj_group(1, c, "wv", "v", s1["v"])
                        vproj_group(1, c, "wvc", "vc", s1["vc"])
                    s1["amix"], s1["vmix"] = make_mixes(1, s1["v"], s1["vc"])
                    alloc_item(1)

            # C: leftover item-0 groups keep the PE fed while the second
            # normalize half of item 1 drains, then item 1's wide groups
            for g in range(10, 16):
                outproj_group(0, g // 2, g % 2, state[0]["ou"],
                              ptag=("t1" if g % 2 == 0 else "t2"), pbufs=1)
            for c in range(NCH):
                outproj_wide(1, c, state[1]["ou"])

    nc.compile()
    return nc


def _get_built():
    global _BUILT
    if _BUILT is None:
        _BUILT = _build()
    return _BUILT


def kernel(x, Wq, Wk, Wv, Wvc, Wp, bp):
    global LAST_RESULTS
    from concourse.bass_utils import run_bass_kernel_spmd

    x = np.asarray(x, dtype=np.float32)
    bf = ml_dtypes.bfloat16
    xT = np.ascontiguousarray(x.transpose(0, 2, 1)).astype(bf)      # (B, C, N)
    ws = {
        "wq": np.asarray(Wq, dtype=np.float32).astype(bf),
        "wk": np.asarray(Wk, dtype=np.float32).astype(bf),
        "wv": np.asarray(Wv, dtype=np.float32).astype(bf),
        "wvc": np.asarray(Wvc, dtype=np.float32).astype(bf),
        "wp": np.asarray(Wp, dtype=np.float32).astype(bf),
    }
    bias = np.ascontiguousarray(
        np.broadcast_to(np.asarray(bp, dtype=np.float32), (128, C))
    )

    if TRACE:
        _install_trace_shim()

    nc = _get_built()
    in_maps = []
    for i in range(N_CORES):
        m = {"xT": np.ascontiguousarray(xT[i * BPC:(i + 1) * BPC]), "bias": bias}
        m.update(ws)
        in_maps.append(m)

    res = run_bass_kernel_spmd(nc, in_maps, list(range(N_CORES)), trace=TRACE,
                               stitch_traces=False)
    LAST_RESULTS = res
    out = np.concatenate([res.results[i]["out"] for i in range(N_CORES)], axis=0)
    return out
